# revision 35
# baseline (speedup 1.0000x reference)
"""Trainium2 Bass kernel for EnhancedGraphAttentionLayer (B=1, N=1024, D=64).

Sharding: destination-node rows split across 8 cores (128 rows each).
Each core is fully independent (no collectives): it holds h replicated and
computes its 128 rows of scores/softmax/attention locally.

Row-paired formulation (2 destination rows per matmul stream):
  pre_i = 0.8*A_e^T relu(ej + ei_i + b) + V + u_i, with V = Mv^T hT constant
  across i (Mv = W@A_j + 0.2*E_j@A_e) and u_i = A_i^T Wh_i + b1
  + 0.2*A_e^T(ei_i + b) per-row.
  - stage1 (ACT): relu(ejT2 + eib2_col) -> rhs1 [128, N]; ejT2 holds ej^T
    duplicated in both partition halves, so one op covers rows i and i+1.
  - mm1 (PE): blockdiag(0.8A_e, 0.8A_e)^T @ rhs1 -> psum1 [128, N]: rows
    0:64 belong to row i, 64:128 to row i+1.
  - stage2 (custom DVE): rhs2 = LeakyRelu(psum1 + V2 + u2_col), V2 = [V; V].
  - mm2 (PE): paired one-hot lhsT accumulates e rows for i and i+1 into a
    32-row PSUM bank (w2 columns at bank rows 2q / 2q+1).
  e = w2^T LeakyRelu(pre) exactly (u rides inside the nonlinearity).
  Mask: e += (adj-1)*1e9, then softmax, attn @ Wh + h, LayerNorm.
The loop is software-pipelined (mm1 of pair p+1 issues before mm2 of pair
p) so the PE never waits on stage2.
"""
import sys
import numpy as np

if "/opt/trn_rl_repo" not in sys.path:
    sys.path.insert(0, "/opt/trn_rl_repo")

import ml_dtypes
import concourse.bass as bass
import concourse.bacc as bacc
import concourse.mybir as mybir
import concourse.tile as tile
from concourse.bass_utils import run_bass_kernel_spmd
from concourse.dve_spec import Spec, Src0, Src1, C0, C2, lower, maxx
from concourse.dve_uop import DveOpSpec
from concourse.dve_ops import (DveOp, OPS, CUSTOM_DVE_SPECS,
                               _SUB_OPCODE_FOR_NAME, _CUSTOM_DVE_ROW_BASE)

F32 = mybir.dt.float32
F32R = mybir.dt.float32r
BF16 = mybir.dt.bfloat16
AF = mybir.ActivationFunctionType
ALU = mybir.AluOpType
AX = mybir.AxisListType

# Pin Relu/Exp/Ln to the one act-table set that holds all three
# ("natural_log_exp_and_others"), so the kernel needs a single
# ACT_TABLE_LOAD instead of mid-kernel table swaps (~1.3 us each + drain).
import concourse.hw_specs as _hw
import concourse.bacc as _bacc_mod


def _pin_act_tables():
    if getattr(_hw, "_act_tables_pinned", False):
        return
    orig = _hw.get_activation_tables

    import functools

    @functools.cache
    def pinned(arch):
        t = dict(orig(arch))
        keep = "natural_log_exp_and_others"
        if keep not in t:
            return t
        pin_funcs = {AF.Relu, AF.Exp, AF.Ln} & t[keep]
        return {name: (funcs if name == keep else funcs - pin_funcs)
                for name, funcs in t.items()}

    _hw.get_activation_tables = pinned
    _bacc_mod.get_activation_tables = pinned
    _hw._act_tables_pinned = True


_pin_act_tables()

N = 1024
D = 64
NCORES = 8
R = N // NCORES          # 128 rows per core
P = R // 2               # 64 row-pairs per core
ALPHA = 0.2
LN_EPS = 1e-5

_CACHE = {}


def _register_lrelu_vb():
    """out = LeakyRelu(in0 + in1 + s0) with slope imm2, registered at runtime."""
    name = "LRELU_VB_ANT"
    if name in _SUB_OPCODE_FOR_NAME:
        return next(op for op in OPS if op.name == name)
    y = Src0 + Src1 + C0
    spec = Spec(
        body=maxx(y, y * C2),
        reference=lambda in0, in1, s0, s1, imm2: np.maximum(
            in0 + in1 + s0, (in0 + in1 + s0) * imm2),
    )
    shas = {}
    for ver in ("v3", "v4"):
        shas[ver] = DveOpSpec(name=name, uops=lower(spec, ver=ver), opcode=0,
                              rd1_en=True).sha(ver)
    op = DveOp(name, spec, subdim=False, uops_sha=shas)
    OPS.append(op)
    row = _CUSTOM_DVE_ROW_BASE + len(OPS) - 1
    assert row < 0x20
    _SUB_OPCODE_FOR_NAME[name] = row
    CUSTOM_DVE_SPECS[name] = spec
    return op


def _build_program():
    lrelu_vb = _register_lrelu_vb()
    nc = bacc.Bacc("TRN2", target_bir_lowering=False, debug=False,
                   num_devices=NCORES)

    def din(name, shape, dt):
        return nc.dram_tensor(name, shape, dt, kind="ExternalInput").ap()

    hT_f = din("hT_f", [D, N], F32R)
    hTr2 = din("hTr2", [2 * D, P], F32)
    hrows = din("hrows", [R, D], F32)
    adjbig = din("adjbig", [R, N], F32)
    lhsT1p = din("lhsT1p", [2 * D, 2 * D], BF16)   # blockdiag(0.8Ae, 0.8Ae)
    lhsT2u = din("lhsT2u", [2 * D, 16 * 32], BF16)  # paired one-hot w2 cols
    ej2w = din("ej2w", [D, 2 * D], F32R)           # [Ej | Ej]
    v2w = din("v2w", [D, 2 * D], F32R)             # [Mv | Mv]
    Ei2 = din("Ei2", [2 * D, 2 * D], F32)          # blockdiag(Ei, Ei)
    W2 = din("W2", [2 * D, 2 * D], F32)            # blockdiag(W, W)
    Ai2 = din("Ai2", [2 * D, 2 * D], F32)          # blockdiag(Ai, Ai)
    Ae2 = din("Ae2", [2 * D, 2 * D], F32)          # blockdiag(Ae, Ae)
    Wm = din("Wm", [D, D], F32R)
    b12col = din("b12col", [2 * D, 1], F32)
    eb2col = din("eb2col", [2 * D, 1], F32)
    iden = din("iden", [128, 128], BF16)
    lngr = din("lngr", [R, D], F32)
    lnbr = din("lnbr", [R, D], F32)
    out_d = nc.dram_tensor("out", [R, D], F32, kind="ExternalOutput").ap()

    with tile.TileContext(nc) as tc, \
         tc.tile_pool(name="static", bufs=1) as sp:
        # ---------------- static SBUF tiles ----------------
        hT_sb = sp.tile([D, N], F32R, name="hT_sb", tag="hT_sb")
        hTr2_sb = sp.tile([2 * D, P], F32, name="hTr2_sb", tag="hTr2_sb")
        hrows_sb = sp.tile([R, D], F32, name="hrows_sb", tag="hrows_sb")
        adjb_sb = sp.tile([R, N], F32, name="adjb_sb", tag="adjb_sb")
        lhsT1p_sb = sp.tile([2 * D, 2 * D], BF16, name="lhsT1p_sb", tag="l1")
        lhsT2u_sb = sp.tile([2 * D, 16 * 32], BF16, name="lhsT2u_sb", tag="l2")
        ej2w_sb = sp.tile([D, 2 * D], F32R, name="ej2w_sb", tag="ej2w")
        v2w_sb = sp.tile([D, 2 * D], F32R, name="v2w_sb", tag="v2w")
        Ei2_sb = sp.tile([2 * D, 2 * D], F32, name="Ei2_sb", tag="Ei2")
        W2_sb = sp.tile([2 * D, 2 * D], F32, name="W2_sb", tag="W2")
        Ai2_sb = sp.tile([2 * D, 2 * D], F32, name="Ai2_sb", tag="Ai2")
        Ae2_sb = sp.tile([2 * D, 2 * D], F32, name="Ae2_sb", tag="Ae2")
        Wm_sb = sp.tile([D, D], F32R, name="Wm_sb", tag="Wm")
        b12_sb = sp.tile([2 * D, 1], F32, name="b12_sb", tag="b12")
        eb2_sb = sp.tile([2 * D, 1], F32, name="eb2_sb", tag="eb2")
        iden_sb = sp.tile([128, 128], BF16, name="iden_sb", tag="iden")
        lngr_sb = sp.tile([R, D], F32, name="lngr_sb", tag="lngr")
        lnbr_sb = sp.tile([R, D], F32, name="lnbr_sb", tag="lnbr")

        ejT2_sb = sp.tile([2 * D, N], BF16, name="ejT2_sb", tag="ejT2")
        V2_sb = sp.tile([2 * D, N], F32, name="V2_sb", tag="V2")
        eib2_sb = sp.tile([2 * D, P], F32, name="eib2_sb", tag="eib2")
        WhTr2_sb = sp.tile([2 * D, P], F32, name="WhTr2_sb", tag="WhTr2")
        q2_sb = sp.tile([2 * D, P], F32, name="q2_sb", tag="q2")
        u2_sb = sp.tile([2 * D, P], F32, name="u2_sb", tag="u2")
        Wh_sb = sp.tile([128, 8 * D], BF16, name="Wh_sb", tag="Wh")
        # 3 slots each: the lag-2 software pipeline keeps 3 pairs in flight
        rhs1_sb = sp.tile([2 * D, 3 * N], BF16, name="rhs1_sb", tag="rhs1")
        rhs2_sb = sp.tile([2 * D, 3 * N], BF16, name="rhs2_sb", tag="rhs2")
        e_sb = sp.tile([R, N], F32, name="e_sb", tag="e_sb")
        ex_sb = sp.tile([R, N], BF16, name="ex_sb", tag="ex_sb")
        attnT_sb = sp.tile([128, N], BF16, name="attnT_sb", tag="attnT")
        scr_sb = sp.tile([1, 8], F32, name="scr_sb", tag="scr")
        red_sb = sp.tile([R, 8], F32, name="red_sb", tag="red")
        hp_sb = sp.tile([R, D], F32, name="hp_sb", tag="hp")
        xm_sb = sp.tile([R, D], F32, name="xm_sb", tag="xm")
        sq_sb = sp.tile([R, D], F32, name="sq_sb", tag="sq")
        o_sb = sp.tile([R, D], F32, name="o_sb", tag="o")

        # ------------- load inputs (critical-path tensors first) -------------
        nc.sync.dma_start(hT_sb[:], hT_f)
        nc.sync.dma_start(ej2w_sb[:], ej2w)
        nc.sync.dma_start(hTr2_sb[:], hTr2)
        nc.sync.dma_start(Ei2_sb[:], Ei2)
        nc.sync.dma_start(eb2_sb[:], eb2col)
        nc.sync.dma_start(lhsT1p_sb[:], lhsT1p)
        nc.sync.dma_start(lhsT2u_sb[:], lhsT2u)
        nc.sync.dma_start(v2w_sb[:], v2w)
        nc.sync.dma_start(W2_sb[:], W2)
        nc.sync.dma_start(Ai2_sb[:], Ai2)
        nc.sync.dma_start(Ae2_sb[:], Ae2)
        nc.sync.dma_start(Wm_sb[:], Wm)
        nc.sync.dma_start(b12_sb[:], b12col)
        nc.sync.dma_start(adjb_sb[:], adjbig)
        nc.sync.dma_start(hrows_sb[:], hrows)
        nc.sync.dma_start(iden_sb[:], iden)
        nc.sync.dma_start(lngr_sb[:], lngr)
        nc.sync.dma_start(lnbr_sb[:], lnbr)

        # warm ACT table sets early (exp/ln)
        nc.vector.memset(scr_sb[:], 1.0)
        nc.scalar.activation(scr_sb[0:1, 0:1], scr_sb[0:1, 1:2], AF.Exp)
        nc.scalar.activation(scr_sb[0:1, 2:3], scr_sb[0:1, 3:4], AF.Ln)

        # ---------------- setup math ----------------
        with tc.tile_pool(name="ps_setup", bufs=1, space="PSUM") as psp:
            # ejT2 = [Ej|Ej]^T hT (bf16), V2 = [Mv|Mv]^T hT (f32)
            for jh in range(2):
                ej_ps = psp.tile([2 * D, 512], F32, name="ej_ps", tag="big",
                                 bufs=2)
                nc.tensor.matmul(ej_ps[:], ej2w_sb[:],
                                 hT_sb[:, jh * 512:(jh + 1) * 512])
                nc.vector.tensor_copy(ejT2_sb[:, jh * 512:(jh + 1) * 512],
                                      ej_ps[:])
            # eib2 = blockdiag(Ei,Ei)^T hTr2 + eb2 (gates stage1(0) - early)
            eib_ps = psp.tile([2 * D, P], F32, name="eib_ps", tag="small",
                              bufs=2)
            nc.tensor.matmul(eib_ps[:], Ei2_sb[:], hTr2_sb[:])
            nc.vector.tensor_scalar(eib2_sb[:], eib_ps[:], eb2_sb[:], None,
                                    op0=ALU.add)
            for jh in range(2):
                v_ps = psp.tile([2 * D, 512], F32, name="v_ps", tag="big",
                                bufs=2)
                nc.tensor.matmul(v_ps[:], v2w_sb[:],
                                 hT_sb[:, jh * 512:(jh + 1) * 512])
                nc.vector.tensor_copy(V2_sb[:, jh * 512:(jh + 1) * 512],
                                      v_ps[:])
            # WhTr2 = blockdiag(W,W)^T hTr2 (paired row projections)
            whtr_ps = psp.tile([2 * D, P], F32, name="whtr_ps", tag="small",
                               bufs=2)
            nc.tensor.matmul(whtr_ps[:], W2_sb[:], hTr2_sb[:])
            nc.vector.tensor_copy(WhTr2_sb[:], whtr_ps[:])
            # q2 = blockdiag(Ai,Ai)^T WhTr2 + b12
            q_ps = psp.tile([2 * D, P], F32, name="q_ps", tag="small", bufs=2)
            nc.tensor.matmul(q_ps[:], Ai2_sb[:], WhTr2_sb[:])
            nc.vector.tensor_scalar(q2_sb[:], q_ps[:], b12_sb[:], None,
                                    op0=ALU.add)
            # u2 = q2 + 0.2 * blockdiag(Ae,Ae)^T eib2
            z_ps = psp.tile([2 * D, P], F32, name="z_ps", tag="small", bufs=2)
            nc.tensor.matmul(z_ps[:], Ae2_sb[:], eib2_sb[:])
            nc.vector.scalar_tensor_tensor(
                u2_sb[:], z_ps[:], ALPHA, q2_sb[:], op0=ALU.mult, op1=ALU.add)

        # ---- main loop over 64 row-pairs (lag-2 software pipeline) ----
        # PE program order: mm1(p+2) precedes mm2(p), so stage2(p) on the DVE
        # has two full mm1 windows to finish before the PE needs its output.
        def stage1(p):
            buf = p % 3
            nc.scalar.activation(rhs1_sb[:, buf * N:(buf + 1) * N],
                                 ejT2_sb[:], AF.Relu,
                                 bias=eib2_sb[:, p:p + 1], scale=1.0)

        with tc.tile_pool(name="ps_mm1", bufs=2, space="PSUM") as pmm1, \
             tc.tile_pool(name="ps_e", bufs=4, space="PSUM") as pe:
            psum1 = [None, None]

            def mm1(p):
                s = p % 3
                psum1[p % 2] = pmm1.tile([2 * D, N], F32, name="psum1",
                                         tag="psum1")
                for jh in range(2):
                    nc.tensor.matmul(
                        psum1[p % 2][:, jh * 512:(jh + 1) * 512],
                        lhsT1p_sb[:],
                        rhs1_sb[:, s * N + jh * 512: s * N + (jh + 1) * 512])

            def stage2(p):
                s = p % 3
                nc.vector._custom_dve(
                    lrelu_vb,
                    out=rhs2_sb[:, s * N:(s + 1) * N],
                    in0=psum1[p % 2][:], in1=V2_sb[:],
                    s0=u2_sb[:, p:p + 1], imm2=ALPHA)

            banks = {}      # group -> [bankE_jh0, bankE_jh1]
            stage1(0)
            mm1(0)
            stage1(1)
            mm1(1)
            stage2(0)
            for p in range(P):
                q = p % 16
                grp = p // 16
                buf = p % 3
                if q == 0:
                    banks[grp] = [pe.tile([32, 512], F32, name="bankE",
                                          tag="bankE") for _ in range(2)]
                if p + 2 < P:
                    stage1(p + 2)
                    mm1(p + 2)
                if p + 1 < P:
                    stage2(p + 1)
                # score matmul: accumulate e rows (2q, 2q+1) into banks
                for jh in range(2):
                    nc.tensor.matmul(
                        banks[grp][jh][:],
                        lhsT2u_sb[:, q * 32:(q + 1) * 32],
                        rhs2_sb[:, buf * N + jh * 512: buf * N + (jh + 1) * 512],
                        start=(q == 0), stop=(q == 15))
                if q == 15:
                    # drain bank -> e_sb with the adjacency mask folded in
                    for jh in range(2):
                        dst = e_sb[grp * 32:(grp + 1) * 32,
                                   jh * 512:(jh + 1) * 512]
                        nc.vector.tensor_tensor(
                            dst, banks[grp][jh][:],
                            adjb_sb[grp * 32:(grp + 1) * 32,
                                    jh * 512:(jh + 1) * 512],
                            op=ALU.add)

        # ---------------- softmax (e_sb is already masked) ----------------
        nc.vector.reduce_max(red_sb[:, 0:1], e_sb[:], axis=AX.X)
        nc.vector.tensor_scalar(red_sb[:, 1:2], red_sb[:, 0:1], -1.0, None,
                                op0=ALU.mult)

        # Wh node-major [128, 64] x 8 tiles — emitted after the main loop so
        # the PE computes it during the softmax reductions (Wh is only
        # needed by the final attn @ Wh).
        with tc.tile_pool(name="ps_wh", bufs=2, space="PSUM") as pw:
            for t in range(8):
                wh_ps = pw.tile([128, D], F32, name="wh_ps", tag="wh", bufs=2)
                nc.tensor.matmul(wh_ps[:], hT_sb[:, t * 128:(t + 1) * 128],
                                 Wm_sb[:])
                nc.vector.tensor_copy(Wh_sb[:, t * D:(t + 1) * D], wh_ps[:])

        # exp in two halves so the first transposes can start earlier
        for jh in range(2):
            nc.scalar.activation(ex_sb[:, jh * 512:(jh + 1) * 512],
                                 e_sb[:, jh * 512:(jh + 1) * 512], AF.Exp,
                                 bias=red_sb[:, 1:2], scale=1.0,
                                 accum_out=red_sb[:, 2 + jh:3 + jh])
        nc.vector.tensor_tensor(red_sb[:, 2:3], red_sb[:, 2:3],
                                red_sb[:, 3:4], op=ALU.add)
        nc.vector.reciprocal(red_sb[:, 4:5], red_sb[:, 2:3])

        # ------- h' = (ex @ Wh) * recip + h ; LayerNorm (normalize late) ----
        with tc.tile_pool(name="ps_fin", bufs=4, space="PSUM") as pf:
            for t in range(8):
                tp_ps = pf.tile([128, 128], BF16, name="tp_ps", tag="tp")
                nc.tensor.transpose(tp_ps[:], ex_sb[:, t * 128:(t + 1) * 128],
                                    iden_sb[:])
                nc.vector.tensor_copy(attnT_sb[:, t * 128:(t + 1) * 128],
                                      tp_ps[:])
            hp_ps = pf.tile([R, D], F32, name="hp_ps", bufs=1)
            for t in range(8):
                nc.tensor.matmul(hp_ps[:], attnT_sb[:, t * 128:(t + 1) * 128],
                                 Wh_sb[:, t * D:(t + 1) * D],
                                 start=(t == 0), stop=(t == 7))
            nc.vector.tensor_scalar(hp_sb[:], hp_ps[:], red_sb[:, 4:5], None,
                                    op0=ALU.mult)
            nc.vector.tensor_tensor(hp_sb[:], hp_sb[:], hrows_sb[:],
                                    op=ALU.add)

        nc.vector.reduce_sum(red_sb[:, 5:6], hp_sb[:], axis=AX.X)
        nc.vector.tensor_scalar(red_sb[:, 6:7], red_sb[:, 5:6], 1.0 / D, None,
                                op0=ALU.mult)
        nc.vector.tensor_scalar(xm_sb[:], hp_sb[:], red_sb[:, 6:7], None,
                                op0=ALU.subtract)
        nc.vector.tensor_tensor(sq_sb[:], xm_sb[:], xm_sb[:], op=ALU.mult)
        nc.vector.reduce_sum(red_sb[:, 7:8], sq_sb[:], axis=AX.X)
        # rstd = exp(-0.5 * ln(var + eps))
        nc.vector.tensor_scalar(red_sb[:, 7:8], red_sb[:, 7:8], 1.0 / D,
                                LN_EPS, op0=ALU.mult, op1=ALU.add)
        nc.scalar.activation(red_sb[:, 0:1], red_sb[:, 7:8], AF.Ln)
        nc.scalar.activation(red_sb[:, 0:1], red_sb[:, 0:1], AF.Exp,
                             bias=0.0, scale=-0.5)
        nc.vector.tensor_scalar(xm_sb[:], xm_sb[:], red_sb[:, 0:1], None,
                                op0=ALU.mult)
        nc.vector.tensor_tensor(o_sb[:], xm_sb[:], lngr_sb[:], op=ALU.mult)
        nc.vector.tensor_tensor(o_sb[:], o_sb[:], lnbr_sb[:], op=ALU.add)
        nc.sync.dma_start(out_d, o_sb[:])

    nc.compile()
    return nc


def _host_prep(inputs):
    h = np.asarray(inputs["h"], np.float32)[0]            # [N, D]
    adj = np.asarray(inputs["adj"])[0]                    # [N, N] int32
    W = np.asarray(inputs["W"], np.float32)
    attn_w1 = np.asarray(inputs["attn_w1"], np.float32)
    attn_b1 = np.asarray(inputs["attn_b1"], np.float32)
    attn_w2 = np.asarray(inputs["attn_w2"], np.float32)
    edge_w = np.asarray(inputs["edge_w"], np.float32)
    edge_b = np.asarray(inputs["edge_b"], np.float32)
    ln_g = np.asarray(inputs["ln_g"], np.float32)
    ln_b = np.asarray(inputs["ln_b"], np.float32)

    A_i, A_j, A_e = attn_w1[:D], attn_w1[D:2 * D], attn_w1[2 * D:]
    E_i, E_j = edge_w[:D], edge_w[D:]
    w2 = attn_w2[:, 0]

    hT = np.ascontiguousarray(h.T)                        # [D, N]
    Mv = W @ A_j + ALPHA * (E_j @ A_e)

    def blockdiag(M):
        Z = np.zeros((2 * D, 2 * D), np.float32)
        Z[:D, :D] = M
        Z[D:, D:] = M
        return Z

    lhsT1p = blockdiag(0.8 * A_e)
    lhsT2u = np.zeros((2 * D, 16 * 32), np.float32)
    for q in range(16):
        lhsT2u[:D, q * 32 + 2 * q] = w2
        lhsT2u[D:, q * 32 + 2 * q + 1] = w2

    rep = {
        "hT_f": hT,
        "lhsT1p": lhsT1p.astype(ml_dtypes.bfloat16),
        "lhsT2u": lhsT2u.astype(ml_dtypes.bfloat16),
        "ej2w": np.ascontiguousarray(np.concatenate([E_j, E_j], axis=1)),
        "v2w": np.ascontiguousarray(np.concatenate([Mv, Mv], axis=1)),
        "Ei2": blockdiag(E_i),
        "W2": blockdiag(W),
        "Ai2": blockdiag(A_i),
        "Ae2": blockdiag(A_e),
        "Wm": W,
        "b12col": np.concatenate([attn_b1, attn_b1])[:, None].copy(),
        "eb2col": np.concatenate([edge_b, edge_b])[:, None].copy(),
        "iden": np.eye(128, dtype=np.float32).astype(ml_dtypes.bfloat16),
        "lngr": np.broadcast_to(ln_g, (R, D)).copy(),
        "lnbr": np.broadcast_to(ln_b, (R, D)).copy(),
    }
    in_maps = []
    for c in range(NCORES):
        rows = slice(c * R, (c + 1) * R)
        hTc = hT[:, rows]                                 # [D, R]
        hTr2 = np.concatenate([hTc[:, 0::2], hTc[:, 1::2]], axis=0)  # [2D, P]
        m = dict(rep)
        m["hTr2"] = np.ascontiguousarray(hTr2)
        m["hrows"] = np.ascontiguousarray(h[rows])
        m["adjbig"] = ((adj[rows] - 1.0) * 1e9).astype(np.float32)
        in_maps.append(m)
    return in_maps


def _get_nc():
    if "nc" not in _CACHE:
        _CACHE["nc"] = _build_program()
    return _CACHE["nc"]


def kernel(**inputs) -> np.ndarray:
    nc = _get_nc()
    in_maps = _host_prep(inputs)
    res = run_bass_kernel_spmd(nc, in_maps, list(range(NCORES))).results
    out = np.concatenate([res[c]["out"] for c in range(NCORES)], axis=0)
    # rows were processed pair-interleaved: out row order is [0,2,4,...,1,3,...]
    # per core? No: bank row 2q <- pair q row i=2q (even), 2q+1 <- odd. e_sb
    # rows are already in natural order (row 2q of a 32-row group is node
    # grp*32+2q). Natural order preserved.
    return out[None].astype(np.float32)


# revision 42
# speedup vs baseline: 1.1777x; 1.1777x over previous
"""Trainium2 Bass kernel for EnhancedGraphAttentionLayer (B=1, N=1024, D=64).

Sharding: destination-node rows split across 8 cores (128 rows each).
Each core is fully independent (no collectives): it holds h replicated and
computes its 128 rows of scores/softmax/attention locally.

Row-paired formulation (2 destination rows per matmul stream):
  pre_i = 0.8*A_e^T relu(ej + ei_i + b) + V + u_i, with V = Mv^T hT constant
  across i (Mv = W@A_j + 0.2*E_j@A_e) and u_i = A_i^T Wh_i + b1
  + 0.2*A_e^T(ei_i + b) per-row.
  - stage1 (ACT): relu(ejT2 + eib2_col) -> rhs1 [128, N]; ejT2 holds ej^T
    duplicated in both partition halves, so one op covers rows i and i+1.
  - mm1 (PE): blockdiag(0.8A_e, 0.8A_e)^T @ rhs1 -> psum1 [128, N]: rows
    0:64 belong to row i, 64:128 to row i+1.
  - stage2 (custom DVE): rhs2 = LeakyRelu(psum1 + V2 + u2_col), V2 = [V; V].
  - mm2 (PE): paired one-hot lhsT accumulates e rows for i and i+1 into a
    32-row PSUM bank (w2 columns at bank rows 2q / 2q+1).
  e = w2^T LeakyRelu(pre) exactly (u rides inside the nonlinearity).
  Mask: e += (adj-1)*1e9, then softmax, attn @ Wh + h, LayerNorm.
The loop is software-pipelined with lag 2 (mm1 of pair p+2 issues before
mm2 of pair p) so the PE never waits on the DVE stage2. Wh is computed
after the main loop (PE fills the softmax-reduction window); the final
ex-transpose + attn @ Wh stack runs in bf16 and normalization by the
softmax denominator happens after the matmul, on [R, D] instead of [R, N].
All activations are pinned to one ACT table set (exp/ln/relu) so the
kernel pays a single ACT_TABLE_LOAD.
"""
import sys
import numpy as np

if "/opt/trn_rl_repo" not in sys.path:
    sys.path.insert(0, "/opt/trn_rl_repo")

import ml_dtypes
import concourse.bass as bass
import concourse.bacc as bacc
import concourse.mybir as mybir
import concourse.tile as tile
from concourse.bass_utils import run_bass_kernel_spmd
from concourse.dve_spec import Spec, Src0, Src1, C0, C2, lower, maxx
from concourse.dve_uop import DveOpSpec
from concourse.dve_ops import (DveOp, OPS, CUSTOM_DVE_SPECS,
                               _SUB_OPCODE_FOR_NAME, _CUSTOM_DVE_ROW_BASE)

F32 = mybir.dt.float32
F32R = mybir.dt.float32r
BF16 = mybir.dt.bfloat16
AF = mybir.ActivationFunctionType
ALU = mybir.AluOpType
AX = mybir.AxisListType

# Pin Relu/Exp/Ln to the one act-table set that holds all three
# ("natural_log_exp_and_others"), so the kernel needs a single
# ACT_TABLE_LOAD instead of mid-kernel table swaps (~1.3 us each + drain).
import concourse.hw_specs as _hw
import concourse.bacc as _bacc_mod


def _pin_act_tables():
    if getattr(_hw, "_act_tables_pinned", False):
        return
    orig = _hw.get_activation_tables

    import functools

    @functools.cache
    def pinned(arch):
        t = dict(orig(arch))
        keep = "natural_log_exp_and_others"
        if keep not in t:
            return t
        pin_funcs = {AF.Relu, AF.Exp, AF.Ln} & t[keep]
        return {name: (funcs if name == keep else funcs - pin_funcs)
                for name, funcs in t.items()}

    _hw.get_activation_tables = pinned
    _bacc_mod.get_activation_tables = pinned
    _hw._act_tables_pinned = True


_pin_act_tables()

N = 1024
D = 64
NCORES = 8
R = N // NCORES          # 128 rows per core
P = R // 2               # 64 row-pairs per core
ALPHA = 0.2
LN_EPS = 1e-5

_CACHE = {}


def _register_dve_op(name, spec):
    if name in _SUB_OPCODE_FOR_NAME:
        return next(op for op in OPS if op.name == name)
    shas = {}
    for ver in ("v3", "v4"):
        shas[ver] = DveOpSpec(name=name, uops=lower(spec, ver=ver), opcode=0,
                              rd1_en=True).sha(ver)
    op = DveOp(name, spec, subdim=False, uops_sha=shas)
    OPS.append(op)
    row = _CUSTOM_DVE_ROW_BASE + len(OPS) - 1
    assert row < 0x20
    _SUB_OPCODE_FOR_NAME[name] = row
    CUSTOM_DVE_SPECS[name] = spec
    return op


def _register_lrelu_vb():
    """out = LeakyRelu(in0 + in1 + s0) with slope imm2, registered at runtime."""
    y = Src0 + Src1 + C0
    return _register_dve_op("LRELU_VB_ANT", Spec(
        body=maxx(y, y * C2),
        reference=lambda in0, in1, s0, s1, imm2: np.maximum(
            in0 + in1 + s0, (in0 + in1 + s0) * imm2),
    ))


def _build_program():
    lrelu_vb = _register_lrelu_vb()
    nc = bacc.Bacc("TRN2", target_bir_lowering=False, debug=False,
                   num_devices=NCORES)

    def din(name, shape, dt):
        return nc.dram_tensor(name, shape, dt, kind="ExternalInput").ap()

    hT_f = din("hT_f", [D, N], F32R)
    hTr2 = din("hTr2", [2 * D, P], F32)
    hrows = din("hrows", [R, D], F32)
    adjbig = din("adjbig", [R, N], F32)
    lhsT1p = din("lhsT1p", [2 * D, 2 * D], BF16)   # blockdiag(0.8Ae, 0.8Ae)
    lhsT2u = din("lhsT2u", [2 * D, 16 * 32], BF16)  # paired one-hot w2 cols
    ej2w = din("ej2w", [D, 2 * D], F32R)           # [Ej | Ej]
    v2w = din("v2w", [D, 2 * D], F32R)             # [Mv | Mv]
    Ei2 = din("Ei2", [2 * D, 2 * D], F32)          # blockdiag(Ei, Ei)
    W2 = din("W2", [2 * D, 2 * D], F32)            # blockdiag(W, W)
    Ai2 = din("Ai2", [2 * D, 2 * D], F32)          # blockdiag(Ai, Ai)
    Ae2 = din("Ae2", [2 * D, 2 * D], F32)          # blockdiag(Ae, Ae)
    Wm = din("Wm", [D, D], F32R)
    b12col = din("b12col", [2 * D, 1], F32)
    eb2col = din("eb2col", [2 * D, 1], F32)
    iden = din("iden", [128, 128], BF16)
    lngr = din("lngr", [R, D], F32)
    lnbr = din("lnbr", [R, D], F32)
    out_d = nc.dram_tensor("out", [R, D], F32, kind="ExternalOutput").ap()

    with tile.TileContext(nc) as tc, \
         tc.tile_pool(name="static", bufs=1) as sp:
        # ---------------- static SBUF tiles ----------------
        hT_sb = sp.tile([D, N], F32R, name="hT_sb", tag="hT_sb")
        hTr2_sb = sp.tile([2 * D, P], F32, name="hTr2_sb", tag="hTr2_sb")
        hrows_sb = sp.tile([R, D], F32, name="hrows_sb", tag="hrows_sb")
        adjb_sb = sp.tile([R, N], F32, name="adjb_sb", tag="adjb_sb")
        lhsT1p_sb = sp.tile([2 * D, 2 * D], BF16, name="lhsT1p_sb", tag="l1")
        lhsT2u_sb = sp.tile([2 * D, 16 * 32], BF16, name="lhsT2u_sb", tag="l2")
        ej2w_sb = sp.tile([D, 2 * D], F32R, name="ej2w_sb", tag="ej2w")
        v2w_sb = sp.tile([D, 2 * D], F32R, name="v2w_sb", tag="v2w")
        Ei2_sb = sp.tile([2 * D, 2 * D], F32, name="Ei2_sb", tag="Ei2")
        W2_sb = sp.tile([2 * D, 2 * D], F32, name="W2_sb", tag="W2")
        Ai2_sb = sp.tile([2 * D, 2 * D], F32, name="Ai2_sb", tag="Ai2")
        Ae2_sb = sp.tile([2 * D, 2 * D], F32, name="Ae2_sb", tag="Ae2")
        Wm_sb = sp.tile([D, D], F32R, name="Wm_sb", tag="Wm")
        b12_sb = sp.tile([2 * D, 1], F32, name="b12_sb", tag="b12")
        eb2_sb = sp.tile([2 * D, 1], F32, name="eb2_sb", tag="eb2")
        iden_sb = sp.tile([128, 128], BF16, name="iden_sb", tag="iden")
        lngr_sb = sp.tile([R, D], F32, name="lngr_sb", tag="lngr")
        lnbr_sb = sp.tile([R, D], F32, name="lnbr_sb", tag="lnbr")

        ejT2_sb = sp.tile([2 * D, N], BF16, name="ejT2_sb", tag="ejT2")
        V2_sb = sp.tile([2 * D, N], F32, name="V2_sb", tag="V2")
        eib2_sb = sp.tile([2 * D, P], F32, name="eib2_sb", tag="eib2")
        WhTr2_sb = sp.tile([2 * D, P], F32, name="WhTr2_sb", tag="WhTr2")
        q2_sb = sp.tile([2 * D, P], F32, name="q2_sb", tag="q2")
        u2_sb = sp.tile([2 * D, P], F32, name="u2_sb", tag="u2")
        Wh_sb = sp.tile([128, 8 * D], BF16, name="Wh_sb", tag="Wh")
        # 3 slots each: the lag-2 software pipeline keeps 3 pairs in flight
        rhs1_sb = sp.tile([2 * D, 3 * N], BF16, name="rhs1_sb", tag="rhs1")
        rhs2_sb = sp.tile([2 * D, 3 * N], BF16, name="rhs2_sb", tag="rhs2")
        e_sb = sp.tile([R, N], F32, name="e_sb", tag="e_sb")
        ex_sb = sp.tile([R, N], BF16, name="ex_sb", tag="ex_sb")
        attnT_sb = sp.tile([128, N], BF16, name="attnT_sb", tag="attnT")
        scr_sb = sp.tile([1, 8], F32, name="scr_sb", tag="scr")
        red_sb = sp.tile([R, 10], F32, name="red_sb", tag="red")
        hp_sb = sp.tile([R, D], F32, name="hp_sb", tag="hp")
        xm_sb = sp.tile([R, D], F32, name="xm_sb", tag="xm")
        sq_sb = sp.tile([R, D], F32, name="sq_sb", tag="sq")
        o_sb = sp.tile([R, D], F32, name="o_sb", tag="o")

        # ------------- load inputs (critical-path tensors first) -------------
        nc.sync.dma_start(hT_sb[:], hT_f)
        nc.sync.dma_start(ej2w_sb[:], ej2w)
        nc.sync.dma_start(hTr2_sb[:], hTr2)
        nc.sync.dma_start(Ei2_sb[:], Ei2)
        nc.sync.dma_start(eb2_sb[:], eb2col)
        nc.sync.dma_start(lhsT1p_sb[:], lhsT1p)
        nc.sync.dma_start(lhsT2u_sb[:], lhsT2u)
        nc.sync.dma_start(v2w_sb[:], v2w)
        nc.sync.dma_start(W2_sb[:], W2)
        nc.sync.dma_start(Ai2_sb[:], Ai2)
        nc.sync.dma_start(Ae2_sb[:], Ae2)
        nc.sync.dma_start(Wm_sb[:], Wm)
        nc.sync.dma_start(b12_sb[:], b12col)
        # late-needed tensors load via the otherwise-idle GpSimd queue
        nc.gpsimd.dma_start(adjb_sb[:], adjbig)
        nc.gpsimd.dma_start(hrows_sb[:], hrows)
        nc.gpsimd.dma_start(iden_sb[:], iden)
        nc.gpsimd.dma_start(lngr_sb[:], lngr)
        nc.gpsimd.dma_start(lnbr_sb[:], lnbr)

        # warm ACT table sets early (exp/ln)
        nc.vector.memset(scr_sb[:], 1.0)
        nc.scalar.activation(scr_sb[0:1, 0:1], scr_sb[0:1, 1:2], AF.Exp)
        nc.scalar.activation(scr_sb[0:1, 2:3], scr_sb[0:1, 3:4], AF.Ln)

        # ---------------- setup math ----------------
        with tc.tile_pool(name="ps_setup", bufs=1, space="PSUM") as psp:
            # ejT2 = [Ej|Ej]^T hT (bf16), V2 = [Mv|Mv]^T hT (f32)
            for jh in range(2):
                ej_ps = psp.tile([2 * D, 512], F32, name="ej_ps", tag="big",
                                 bufs=2)
                nc.tensor.matmul(ej_ps[:], ej2w_sb[:],
                                 hT_sb[:, jh * 512:(jh + 1) * 512])
                nc.vector.tensor_copy(ejT2_sb[:, jh * 512:(jh + 1) * 512],
                                      ej_ps[:])
            # eib2 = blockdiag(Ei,Ei)^T hTr2 + eb2 (gates stage1(0) - early)
            eib_ps = psp.tile([2 * D, P], F32, name="eib_ps", tag="small",
                              bufs=2)
            nc.tensor.matmul(eib_ps[:], Ei2_sb[:], hTr2_sb[:])
            nc.vector.tensor_scalar(eib2_sb[:], eib_ps[:], eb2_sb[:], None,
                                    op0=ALU.add)
            for jh in range(2):
                v_ps = psp.tile([2 * D, 512], F32, name="v_ps", tag="big",
                                bufs=2)
                nc.tensor.matmul(v_ps[:], v2w_sb[:],
                                 hT_sb[:, jh * 512:(jh + 1) * 512])
                nc.vector.tensor_copy(V2_sb[:, jh * 512:(jh + 1) * 512],
                                      v_ps[:])
            # WhTr2 = blockdiag(W,W)^T hTr2 (paired row projections)
            whtr_ps = psp.tile([2 * D, P], F32, name="whtr_ps", tag="small",
                               bufs=2)
            nc.tensor.matmul(whtr_ps[:], W2_sb[:], hTr2_sb[:])
            nc.vector.tensor_copy(WhTr2_sb[:], whtr_ps[:])
            # q2 = blockdiag(Ai,Ai)^T WhTr2 + b12
            q_ps = psp.tile([2 * D, P], F32, name="q_ps", tag="small", bufs=2)
            nc.tensor.matmul(q_ps[:], Ai2_sb[:], WhTr2_sb[:])
            nc.vector.tensor_scalar(q2_sb[:], q_ps[:], b12_sb[:], None,
                                    op0=ALU.add)
            # u2 = q2 + 0.2 * blockdiag(Ae,Ae)^T eib2
            z_ps = psp.tile([2 * D, P], F32, name="z_ps", tag="small", bufs=2)
            nc.tensor.matmul(z_ps[:], Ae2_sb[:], eib2_sb[:])
            nc.vector.scalar_tensor_tensor(
                u2_sb[:], z_ps[:], ALPHA, q2_sb[:], op0=ALU.mult, op1=ALU.add)

        # ---- main loop over 64 row-pairs (lag-2 software pipeline) ----
        # PE program order: mm1(p+2) precedes mm2(p), so stage2(p) on the DVE
        # has two full mm1 windows to finish before the PE needs its output.
        def stage1(p):
            buf = p % 3
            nc.scalar.activation(rhs1_sb[:, buf * N:(buf + 1) * N],
                                 ejT2_sb[:], AF.Relu,
                                 bias=eib2_sb[:, p:p + 1], scale=1.0)

        with tc.tile_pool(name="ps_mm1", bufs=2, space="PSUM") as pmm1, \
             tc.tile_pool(name="ps_e", bufs=4, space="PSUM") as pe:
            psum1 = [None, None]

            def mm1(p):
                s = p % 3
                psum1[p % 2] = pmm1.tile([2 * D, N], F32, name="psum1",
                                         tag="psum1")
                for jh in range(2):
                    nc.tensor.matmul(
                        psum1[p % 2][:, jh * 512:(jh + 1) * 512],
                        lhsT1p_sb[:],
                        rhs1_sb[:, s * N + jh * 512: s * N + (jh + 1) * 512])

            def stage2(p):
                s = p % 3
                nc.vector._custom_dve(
                    lrelu_vb,
                    out=rhs2_sb[:, s * N:(s + 1) * N],
                    in0=psum1[p % 2][:], in1=V2_sb[:],
                    s0=u2_sb[:, p:p + 1], imm2=ALPHA)

            banks = {}      # group -> [bankE_jh0, bankE_jh1]
            stage1(0)
            mm1(0)
            stage1(1)
            mm1(1)
            stage2(0)
            for p in range(P):
                q = p % 16
                grp = p // 16
                buf = p % 3
                if q == 0:
                    banks[grp] = [pe.tile([32, 512], F32, name="bankE",
                                          tag="bankE") for _ in range(2)]
                if p + 2 < P:
                    stage1(p + 2)
                    mm1(p + 2)
                if p + 1 < P:
                    stage2(p + 1)
                # score matmul: accumulate e rows (2q, 2q+1) into banks
                for jh in range(2):
                    nc.tensor.matmul(
                        banks[grp][jh][:],
                        lhsT2u_sb[:, q * 32:(q + 1) * 32],
                        rhs2_sb[:, buf * N + jh * 512: buf * N + (jh + 1) * 512],
                        start=(q == 0), stop=(q == 15))
                if q == 15:
                    # drain bank -> e_sb with the adjacency mask folded in
                    for jh in range(2):
                        rows = slice(grp * 32, (grp + 1) * 32)
                        nc.vector.tensor_tensor(
                            e_sb[rows, jh * 512:(jh + 1) * 512],
                            banks[grp][jh][:],
                            adjb_sb[rows, jh * 512:(jh + 1) * 512],
                            op=ALU.add)

        # ---------------- softmax (e_sb is already masked) ----------------
        nc.vector.reduce_max(red_sb[:, 0:1], e_sb[:], axis=AX.X)
        nc.vector.tensor_scalar(red_sb[:, 1:2], red_sb[:, 0:1], -1.0, None,
                                op0=ALU.mult)

        # Wh node-major [128, 64] x 8 tiles — emitted after the main loop so
        # the PE computes it during the softmax reductions (Wh is only
        # needed by the final attn @ Wh).
        with tc.tile_pool(name="ps_wh", bufs=2, space="PSUM") as pw:
            for t in range(8):
                wh_ps = pw.tile([128, D], F32, name="wh_ps", tag="wh", bufs=2)
                nc.tensor.matmul(wh_ps[:], hT_sb[:, t * 128:(t + 1) * 128],
                                 Wm_sb[:])
                nc.vector.tensor_copy(Wh_sb[:, t * D:(t + 1) * D], wh_ps[:])

        # exp in two halves so the first transposes can start earlier
        for jh in range(2):
            nc.scalar.activation(ex_sb[:, jh * 512:(jh + 1) * 512],
                                 e_sb[:, jh * 512:(jh + 1) * 512], AF.Exp,
                                 bias=red_sb[:, 1:2], scale=1.0,
                                 accum_out=red_sb[:, 2 + jh:3 + jh])
        nc.vector.tensor_tensor(red_sb[:, 2:3], red_sb[:, 2:3],
                                red_sb[:, 3:4], op=ALU.add)
        nc.vector.reciprocal(red_sb[:, 4:5], red_sb[:, 2:3])

        # ------- h' = (ex @ Wh) * recip + h ; LayerNorm (normalize late) ----
        with tc.tile_pool(name="ps_fin", bufs=4, space="PSUM") as pf:
            for t in range(8):
                tp_ps = pf.tile([128, 128], BF16, name="tp_ps", tag="tp")
                nc.tensor.transpose(tp_ps[:], ex_sb[:, t * 128:(t + 1) * 128],
                                    iden_sb[:])
                if t % 2 == 0:
                    nc.vector.tensor_copy(attnT_sb[:, t * 128:(t + 1) * 128],
                                          tp_ps[:])
                else:
                    nc.scalar.copy(attnT_sb[:, t * 128:(t + 1) * 128],
                                   tp_ps[:])
            hp_ps = pf.tile([R, D], F32, name="hp_ps", bufs=1)
            for t in range(8):
                nc.tensor.matmul(hp_ps[:], attnT_sb[:, t * 128:(t + 1) * 128],
                                 Wh_sb[:, t * D:(t + 1) * D],
                                 start=(t == 0), stop=(t == 7))
            nc.vector.tensor_scalar(hp_sb[:], hp_ps[:], red_sb[:, 4:5], None,
                                    op0=ALU.mult)
            nc.vector.tensor_tensor(hp_sb[:], hp_sb[:], hrows_sb[:],
                                    op=ALU.add)

        nc.vector.reduce_sum(red_sb[:, 5:6], hp_sb[:], axis=AX.X)
        nc.vector.tensor_scalar(red_sb[:, 6:7], red_sb[:, 5:6], 1.0 / D, None,
                                op0=ALU.mult)
        nc.vector.tensor_scalar(xm_sb[:], hp_sb[:], red_sb[:, 6:7], None,
                                op0=ALU.subtract)
        nc.vector.tensor_tensor(sq_sb[:], xm_sb[:], xm_sb[:], op=ALU.mult)
        nc.vector.reduce_sum(red_sb[:, 7:8], sq_sb[:], axis=AX.X)
        # rstd = exp(-0.5 * ln(var + eps))
        nc.vector.tensor_scalar(red_sb[:, 7:8], red_sb[:, 7:8], 1.0 / D,
                                LN_EPS, op0=ALU.mult, op1=ALU.add)
        nc.scalar.activation(red_sb[:, 0:1], red_sb[:, 7:8], AF.Ln)
        nc.scalar.activation(red_sb[:, 0:1], red_sb[:, 0:1], AF.Exp,
                             bias=0.0, scale=-0.5)
        nc.vector.tensor_scalar(xm_sb[:], xm_sb[:], red_sb[:, 0:1], None,
                                op0=ALU.mult)
        nc.vector.tensor_tensor(o_sb[:], xm_sb[:], lngr_sb[:], op=ALU.mult)
        nc.vector.tensor_tensor(o_sb[:], o_sb[:], lnbr_sb[:], op=ALU.add)
        nc.sync.dma_start(out_d, o_sb[:])

    nc.compile()
    return nc


def _host_prep(inputs):
    h = np.asarray(inputs["h"], np.float32)[0]            # [N, D]
    adj = np.asarray(inputs["adj"])[0]                    # [N, N] int32
    W = np.asarray(inputs["W"], np.float32)
    attn_w1 = np.asarray(inputs["attn_w1"], np.float32)
    attn_b1 = np.asarray(inputs["attn_b1"], np.float32)
    attn_w2 = np.asarray(inputs["attn_w2"], np.float32)
    edge_w = np.asarray(inputs["edge_w"], np.float32)
    edge_b = np.asarray(inputs["edge_b"], np.float32)
    ln_g = np.asarray(inputs["ln_g"], np.float32)
    ln_b = np.asarray(inputs["ln_b"], np.float32)

    A_i, A_j, A_e = attn_w1[:D], attn_w1[D:2 * D], attn_w1[2 * D:]
    E_i, E_j = edge_w[:D], edge_w[D:]
    w2 = attn_w2[:, 0]

    hT = np.ascontiguousarray(h.T)                        # [D, N]
    Mv = W @ A_j + ALPHA * (E_j @ A_e)

    def blockdiag(M):
        Z = np.zeros((2 * D, 2 * D), np.float32)
        Z[:D, :D] = M
        Z[D:, D:] = M
        return Z

    lhsT1p = blockdiag(0.8 * A_e)
    lhsT2u = np.zeros((2 * D, 16 * 32), np.float32)
    for q in range(16):
        lhsT2u[:D, q * 32 + 2 * q] = w2
        lhsT2u[D:, q * 32 + 2 * q + 1] = w2

    rep = {
        "hT_f": hT,
        "lhsT1p": lhsT1p.astype(ml_dtypes.bfloat16),
        "lhsT2u": lhsT2u.astype(ml_dtypes.bfloat16),
        "ej2w": np.ascontiguousarray(np.concatenate([E_j, E_j], axis=1)),
        "v2w": np.ascontiguousarray(np.concatenate([Mv, Mv], axis=1)),
        "Ei2": blockdiag(E_i),
        "W2": blockdiag(W),
        "Ai2": blockdiag(A_i),
        "Ae2": blockdiag(A_e),
        "Wm": W,
        "b12col": np.concatenate([attn_b1, attn_b1])[:, None].copy(),
        "eb2col": np.concatenate([edge_b, edge_b])[:, None].copy(),
        "iden": np.eye(128, dtype=np.float32).astype(ml_dtypes.bfloat16),
        "lngr": np.broadcast_to(ln_g, (R, D)).copy(),
        "lnbr": np.broadcast_to(ln_b, (R, D)).copy(),
    }
    in_maps = []
    for c in range(NCORES):
        rows = slice(c * R, (c + 1) * R)
        hTc = hT[:, rows]                                 # [D, R]
        hTr2 = np.concatenate([hTc[:, 0::2], hTc[:, 1::2]], axis=0)  # [2D, P]
        m = dict(rep)
        m["hTr2"] = np.ascontiguousarray(hTr2)
        m["hrows"] = np.ascontiguousarray(h[rows])
        m["adjbig"] = ((adj[rows] - 1.0) * 1e9).astype(np.float32)
        in_maps.append(m)
    return in_maps


def _get_nc():
    if "nc" not in _CACHE:
        _CACHE["nc"] = _build_program()
    return _CACHE["nc"]


def kernel(**inputs) -> np.ndarray:
    nc = _get_nc()
    in_maps = _host_prep(inputs)
    res = run_bass_kernel_spmd(nc, in_maps, list(range(NCORES))).results
    out = np.concatenate([res[c]["out"] for c in range(NCORES)], axis=0)
    # rows were processed pair-interleaved: out row order is [0,2,4,...,1,3,...]
    # per core? No: bank row 2q <- pair q row i=2q (even), 2q+1 <- odd. e_sb
    # rows are already in natural order (row 2q of a 32-row group is node
    # grp*32+2q). Natural order preserved.
    return out[None].astype(np.float32)


# revision 43
# speedup vs baseline: 1.2286x; 1.0432x over previous
"""Trainium2 Bass kernel for EnhancedGraphAttentionLayer (B=1, N=1024, D=64).

Sharding: destination-node rows split across 8 cores (128 rows each).
Each core is fully independent (no collectives): it holds h replicated and
computes its 128 rows of scores/softmax/attention locally.

Row-paired formulation (2 destination rows per matmul stream):
  pre_i = 0.8*A_e^T relu(ej + ei_i + b) + V + u_i, with V = Mv^T hT constant
  across i (Mv = W@A_j + 0.2*E_j@A_e) and u_i = A_i^T Wh_i + b1
  + 0.2*A_e^T(ei_i + b) per-row.
  - stage1 (ACT): relu(ejT2 + eib2_col) -> rhs1 [128, N]; ejT2 holds ej^T
    duplicated in both partition halves, so one op covers rows i and i+1.
  - mm1 (PE): blockdiag(0.8A_e, 0.8A_e)^T @ rhs1 -> psum1 [128, N]: rows
    0:64 belong to row i, 64:128 to row i+1.
  - stage2 (custom DVE): rhs2 = LeakyRelu(psum1 + V2 + u2_col), V2 = [V; V].
  - mm2 (PE): paired one-hot lhsT accumulates e rows for i and i+1 into a
    32-row PSUM bank (w2 columns at bank rows 2q / 2q+1).
  e = w2^T LeakyRelu(pre) exactly (u rides inside the nonlinearity).
  Mask: e += (adj-1)*1e9, then softmax, attn @ Wh + h, LayerNorm.
The loop is software-pipelined with lag 2 (mm1 of pair p+2 issues before
mm2 of pair p) so the PE never waits on the DVE stage2. Wh is computed
after the main loop (PE fills the softmax-reduction window); the final
ex-transpose + attn @ Wh stack runs in bf16 and normalization by the
softmax denominator happens after the matmul, on [R, D] instead of [R, N].
All activations are pinned to one ACT table set (exp/ln/relu) so the
kernel pays a single ACT_TABLE_LOAD.
"""
import sys
import numpy as np

if "/opt/trn_rl_repo" not in sys.path:
    sys.path.insert(0, "/opt/trn_rl_repo")

import ml_dtypes
import concourse.bass as bass
import concourse.bacc as bacc
import concourse.mybir as mybir
import concourse.tile as tile
from concourse.bass_utils import run_bass_kernel_spmd
from concourse.dve_spec import Spec, Src0, Src1, C0, C2, lower, maxx
from concourse.dve_uop import DveOpSpec
from concourse.dve_ops import (DveOp, OPS, CUSTOM_DVE_SPECS,
                               _SUB_OPCODE_FOR_NAME, _CUSTOM_DVE_ROW_BASE)

F32 = mybir.dt.float32
F32R = mybir.dt.float32r
BF16 = mybir.dt.bfloat16
AF = mybir.ActivationFunctionType
ALU = mybir.AluOpType
AX = mybir.AxisListType

# Pin Relu/Exp/Ln to the one act-table set that holds all three
# ("natural_log_exp_and_others"), so the kernel needs a single
# ACT_TABLE_LOAD instead of mid-kernel table swaps (~1.3 us each + drain).
import concourse.hw_specs as _hw
import concourse.bacc as _bacc_mod


def _pin_act_tables():
    if getattr(_hw, "_act_tables_pinned", False):
        return
    orig = _hw.get_activation_tables

    import functools

    @functools.cache
    def pinned(arch):
        t = dict(orig(arch))
        keep = "natural_log_exp_and_others"
        if keep not in t:
            return t
        pin_funcs = {AF.Relu, AF.Exp, AF.Ln} & t[keep]
        return {name: (funcs if name == keep else funcs - pin_funcs)
                for name, funcs in t.items()}

    _hw.get_activation_tables = pinned
    _bacc_mod.get_activation_tables = pinned
    _hw._act_tables_pinned = True


_pin_act_tables()

N = 1024
D = 64
NCORES = 8
R = N // NCORES          # 128 rows per core
P = R // 2               # 64 row-pairs per core
ALPHA = 0.2
LN_EPS = 1e-5

_CACHE = {}


def _register_dve_op(name, spec):
    if name in _SUB_OPCODE_FOR_NAME:
        return next(op for op in OPS if op.name == name)
    shas = {}
    for ver in ("v3", "v4"):
        shas[ver] = DveOpSpec(name=name, uops=lower(spec, ver=ver), opcode=0,
                              rd1_en=True).sha(ver)
    op = DveOp(name, spec, subdim=False, uops_sha=shas)
    OPS.append(op)
    row = _CUSTOM_DVE_ROW_BASE + len(OPS) - 1
    assert row < 0x20
    _SUB_OPCODE_FOR_NAME[name] = row
    CUSTOM_DVE_SPECS[name] = spec
    return op


def _register_lrelu_vb():
    """out = LeakyRelu(in0 + in1 + s0) with slope imm2, registered at runtime."""
    y = Src0 + Src1 + C0
    return _register_dve_op("LRELU_VB_ANT", Spec(
        body=maxx(y, y * C2),
        reference=lambda in0, in1, s0, s1, imm2: np.maximum(
            in0 + in1 + s0, (in0 + in1 + s0) * imm2),
    ))


def _build_program():
    lrelu_vb = _register_lrelu_vb()
    nc = bacc.Bacc("TRN2", target_bir_lowering=False, debug=False,
                   num_devices=NCORES)

    def din(name, shape, dt):
        return nc.dram_tensor(name, shape, dt, kind="ExternalInput").ap()

    hT_f = din("hT_f", [D, N], F32R)
    hTr2 = din("hTr2", [2 * D, P], F32)
    hrows = din("hrows", [R, D], F32)
    adjbig = din("adjbig", [R, N], F32)
    lhsT1p = din("lhsT1p", [2 * D, 2 * D], BF16)   # blockdiag(0.8Ae, 0.8Ae)
    lhsT2u = din("lhsT2u", [2 * D, 16 * 32], BF16)  # paired one-hot w2 cols
    ej2w = din("ej2w", [D, 2 * D], F32R)           # [Ej | Ej]
    v2w = din("v2w", [D, 2 * D], F32R)             # [Mv | Mv]
    Ei2 = din("Ei2", [2 * D, 2 * D], F32)          # blockdiag(Ei, Ei)
    W2 = din("W2", [2 * D, 2 * D], F32)            # blockdiag(W, W)
    Ai2 = din("Ai2", [2 * D, 2 * D], F32)          # blockdiag(Ai, Ai)
    Ae2 = din("Ae2", [2 * D, 2 * D], F32)          # blockdiag(Ae, Ae)
    Wm = din("Wm", [D, D], F32R)
    b12col = din("b12col", [2 * D, 1], F32)
    eb2col = din("eb2col", [2 * D, 1], F32)
    iden = din("iden", [128, 128], BF16)
    lngr = din("lngr", [R, D], F32)
    lnbr = din("lnbr", [R, D], F32)
    out_d = nc.dram_tensor("out", [R, D], F32, kind="ExternalOutput").ap()

    with tile.TileContext(nc) as tc, \
         tc.tile_pool(name="static", bufs=1) as sp:
        # ---------------- static SBUF tiles ----------------
        hT_sb = sp.tile([D, N], F32R, name="hT_sb", tag="hT_sb")
        hTr2_sb = sp.tile([2 * D, P], F32, name="hTr2_sb", tag="hTr2_sb")
        hrows_sb = sp.tile([R, D], F32, name="hrows_sb", tag="hrows_sb")
        adjb_sb = sp.tile([R, N], F32, name="adjb_sb", tag="adjb_sb")
        lhsT1p_sb = sp.tile([2 * D, 2 * D], BF16, name="lhsT1p_sb", tag="l1")
        lhsT2u_sb = sp.tile([2 * D, 16 * 32], BF16, name="lhsT2u_sb", tag="l2")
        ej2w_sb = sp.tile([D, 2 * D], F32R, name="ej2w_sb", tag="ej2w")
        v2w_sb = sp.tile([D, 2 * D], F32R, name="v2w_sb", tag="v2w")
        Ei2_sb = sp.tile([2 * D, 2 * D], F32, name="Ei2_sb", tag="Ei2")
        W2_sb = sp.tile([2 * D, 2 * D], F32, name="W2_sb", tag="W2")
        Ai2_sb = sp.tile([2 * D, 2 * D], F32, name="Ai2_sb", tag="Ai2")
        Ae2_sb = sp.tile([2 * D, 2 * D], F32, name="Ae2_sb", tag="Ae2")
        Wm_sb = sp.tile([D, D], F32R, name="Wm_sb", tag="Wm")
        b12_sb = sp.tile([2 * D, 1], F32, name="b12_sb", tag="b12")
        eb2_sb = sp.tile([2 * D, 1], F32, name="eb2_sb", tag="eb2")
        iden_sb = sp.tile([128, 128], BF16, name="iden_sb", tag="iden")
        lngr_sb = sp.tile([R, D], F32, name="lngr_sb", tag="lngr")
        lnbr_sb = sp.tile([R, D], F32, name="lnbr_sb", tag="lnbr")

        ejT2_sb = sp.tile([2 * D, N], BF16, name="ejT2_sb", tag="ejT2")
        V2_sb = sp.tile([2 * D, N], F32, name="V2_sb", tag="V2")
        eib2_sb = sp.tile([2 * D, P], F32, name="eib2_sb", tag="eib2")
        WhTr2_sb = sp.tile([2 * D, P], F32, name="WhTr2_sb", tag="WhTr2")
        q2_sb = sp.tile([2 * D, P], F32, name="q2_sb", tag="q2")
        u2_sb = sp.tile([2 * D, P], F32, name="u2_sb", tag="u2")
        Wh_sb = sp.tile([128, 8 * D], BF16, name="Wh_sb", tag="Wh")
        # 3 slots each: the lag-2 software pipeline keeps 3 pairs in flight
        rhs1_sb = sp.tile([2 * D, 3 * N], BF16, name="rhs1_sb", tag="rhs1")
        rhs2_sb = sp.tile([2 * D, 3 * N], BF16, name="rhs2_sb", tag="rhs2")
        e_sb = sp.tile([R, N], F32, name="e_sb", tag="e_sb")
        ex_sb = sp.tile([R, N], BF16, name="ex_sb", tag="ex_sb")
        attnT_sb = sp.tile([128, N], BF16, name="attnT_sb", tag="attnT")
        scr_sb = sp.tile([1, 8], F32, name="scr_sb", tag="scr")
        red_sb = sp.tile([R, 10], F32, name="red_sb", tag="red")
        hp_sb = sp.tile([R, D], F32, name="hp_sb", tag="hp")
        xm_sb = sp.tile([R, D], F32, name="xm_sb", tag="xm")
        sq_sb = sp.tile([R, D], F32, name="sq_sb", tag="sq")
        o_sb = sp.tile([R, D], F32, name="o_sb", tag="o")

        # ------------- load inputs (critical-path tensors first) -------------
        # critical path (gates ej2/eib2/stage1/mm1) on the sync queue alone
        nc.sync.dma_start(hT_sb[:], hT_f)
        nc.sync.dma_start(ej2w_sb[:], ej2w)
        nc.sync.dma_start(hTr2_sb[:], hTr2)
        nc.sync.dma_start(Ei2_sb[:], Ei2)
        nc.sync.dma_start(eb2_sb[:], eb2col)
        nc.sync.dma_start(lhsT1p_sb[:], lhsT1p)
        # everything else via the otherwise-idle GpSimd queue
        nc.gpsimd.dma_start(v2w_sb[:], v2w)
        nc.gpsimd.dma_start(W2_sb[:], W2)
        nc.gpsimd.dma_start(Ai2_sb[:], Ai2)
        nc.gpsimd.dma_start(Ae2_sb[:], Ae2)
        nc.gpsimd.dma_start(b12_sb[:], b12col)
        nc.gpsimd.dma_start(lhsT2u_sb[:], lhsT2u)
        nc.gpsimd.dma_start(Wm_sb[:], Wm)
        nc.gpsimd.dma_start(adjb_sb[:], adjbig)
        nc.gpsimd.dma_start(hrows_sb[:], hrows)
        nc.gpsimd.dma_start(iden_sb[:], iden)
        nc.gpsimd.dma_start(lngr_sb[:], lngr)
        nc.gpsimd.dma_start(lnbr_sb[:], lnbr)

        # warm ACT table sets early (exp/ln)
        nc.vector.memset(scr_sb[:], 1.0)
        nc.scalar.activation(scr_sb[0:1, 0:1], scr_sb[0:1, 1:2], AF.Exp)
        nc.scalar.activation(scr_sb[0:1, 2:3], scr_sb[0:1, 3:4], AF.Ln)

        # ---------------- setup math ----------------
        with tc.tile_pool(name="ps_setup", bufs=1, space="PSUM") as psp:
            # ejT2 = [Ej|Ej]^T hT (bf16), V2 = [Mv|Mv]^T hT (f32)
            for jh in range(2):
                ej_ps = psp.tile([2 * D, 512], F32, name="ej_ps", tag="big",
                                 bufs=2)
                nc.tensor.matmul(ej_ps[:], ej2w_sb[:],
                                 hT_sb[:, jh * 512:(jh + 1) * 512])
                nc.vector.tensor_copy(ejT2_sb[:, jh * 512:(jh + 1) * 512],
                                      ej_ps[:])
            # eib2 = blockdiag(Ei,Ei)^T hTr2 + eb2 (gates stage1(0) - early)
            eib_ps = psp.tile([2 * D, P], F32, name="eib_ps", tag="small",
                              bufs=2)
            nc.tensor.matmul(eib_ps[:], Ei2_sb[:], hTr2_sb[:])
            nc.vector.tensor_scalar(eib2_sb[:], eib_ps[:], eb2_sb[:], None,
                                    op0=ALU.add)
            for jh in range(2):
                v_ps = psp.tile([2 * D, 512], F32, name="v_ps", tag="big",
                                bufs=2)
                nc.tensor.matmul(v_ps[:], v2w_sb[:],
                                 hT_sb[:, jh * 512:(jh + 1) * 512])
                nc.vector.tensor_copy(V2_sb[:, jh * 512:(jh + 1) * 512],
                                      v_ps[:])
            # WhTr2 = blockdiag(W,W)^T hTr2 (paired row projections)
            whtr_ps = psp.tile([2 * D, P], F32, name="whtr_ps", tag="small",
                               bufs=2)
            nc.tensor.matmul(whtr_ps[:], W2_sb[:], hTr2_sb[:])
            nc.vector.tensor_copy(WhTr2_sb[:], whtr_ps[:])
            # q2 = blockdiag(Ai,Ai)^T WhTr2 + b12
            q_ps = psp.tile([2 * D, P], F32, name="q_ps", tag="small", bufs=2)
            nc.tensor.matmul(q_ps[:], Ai2_sb[:], WhTr2_sb[:])
            nc.vector.tensor_scalar(q2_sb[:], q_ps[:], b12_sb[:], None,
                                    op0=ALU.add)
            # u2 = q2 + 0.2 * blockdiag(Ae,Ae)^T eib2
            z_ps = psp.tile([2 * D, P], F32, name="z_ps", tag="small", bufs=2)
            nc.tensor.matmul(z_ps[:], Ae2_sb[:], eib2_sb[:])
            nc.vector.scalar_tensor_tensor(
                u2_sb[:], z_ps[:], ALPHA, q2_sb[:], op0=ALU.mult, op1=ALU.add)

        # ---- main loop over 64 row-pairs (lag-2 software pipeline) ----
        # PE program order: mm1(p+2) precedes mm2(p), so stage2(p) on the DVE
        # has two full mm1 windows to finish before the PE needs its output.
        def stage1(p):
            buf = p % 3
            nc.scalar.activation(rhs1_sb[:, buf * N:(buf + 1) * N],
                                 ejT2_sb[:], AF.Relu,
                                 bias=eib2_sb[:, p:p + 1], scale=1.0)

        with tc.tile_pool(name="ps_mm1", bufs=2, space="PSUM") as pmm1, \
             tc.tile_pool(name="ps_e", bufs=4, space="PSUM") as pe:
            psum1 = [None, None]

            def mm1(p):
                s = p % 3
                psum1[p % 2] = pmm1.tile([2 * D, N], F32, name="psum1",
                                         tag="psum1")
                for jh in range(2):
                    nc.tensor.matmul(
                        psum1[p % 2][:, jh * 512:(jh + 1) * 512],
                        lhsT1p_sb[:],
                        rhs1_sb[:, s * N + jh * 512: s * N + (jh + 1) * 512])

            def stage2(p):
                s = p % 3
                nc.vector._custom_dve(
                    lrelu_vb,
                    out=rhs2_sb[:, s * N:(s + 1) * N],
                    in0=psum1[p % 2][:], in1=V2_sb[:],
                    s0=u2_sb[:, p:p + 1], imm2=ALPHA)

            banks = {}      # group -> [bankE_jh0, bankE_jh1]
            stage1(0)
            mm1(0)
            stage1(1)
            mm1(1)
            stage2(0)
            for p in range(P):
                q = p % 16
                grp = p // 16
                buf = p % 3
                if q == 0:
                    banks[grp] = [pe.tile([32, 512], F32, name="bankE",
                                          tag="bankE") for _ in range(2)]
                if p + 2 < P:
                    stage1(p + 2)
                    mm1(p + 2)
                if p + 1 < P:
                    stage2(p + 1)
                # score matmul: accumulate e rows (2q, 2q+1) into banks
                for jh in range(2):
                    nc.tensor.matmul(
                        banks[grp][jh][:],
                        lhsT2u_sb[:, q * 32:(q + 1) * 32],
                        rhs2_sb[:, buf * N + jh * 512: buf * N + (jh + 1) * 512],
                        start=(q == 0), stop=(q == 15))
                if q == 15:
                    # drain bank -> e_sb on the Scalar engine: the DVE is
                    # pacing stage2, and a DVE drain here stalls the PE
                    # ~1.5us at every group boundary
                    for jh in range(2):
                        rows = slice(grp * 32, (grp + 1) * 32)
                        nc.scalar.copy(e_sb[rows, jh * 512:(jh + 1) * 512],
                                       banks[grp][jh][:])

        # ---------------- mask + softmax ----------------
        nc.vector.tensor_tensor(e_sb[:], e_sb[:], adjb_sb[:], op=ALU.add)
        nc.vector.reduce_max(red_sb[:, 0:1], e_sb[:], axis=AX.X)
        nc.vector.tensor_scalar(red_sb[:, 1:2], red_sb[:, 0:1], -1.0, None,
                                op0=ALU.mult)

        # Wh node-major [128, 64] x 8 tiles — emitted after the main loop so
        # the PE computes it during the softmax reductions (Wh is only
        # needed by the final attn @ Wh).
        with tc.tile_pool(name="ps_wh", bufs=2, space="PSUM") as pw:
            for t in range(8):
                wh_ps = pw.tile([128, D], F32, name="wh_ps", tag="wh", bufs=2)
                nc.tensor.matmul(wh_ps[:], hT_sb[:, t * 128:(t + 1) * 128],
                                 Wm_sb[:])
                nc.vector.tensor_copy(Wh_sb[:, t * D:(t + 1) * D], wh_ps[:])

        # exp in two halves so the first transposes can start earlier
        for jh in range(2):
            nc.scalar.activation(ex_sb[:, jh * 512:(jh + 1) * 512],
                                 e_sb[:, jh * 512:(jh + 1) * 512], AF.Exp,
                                 bias=red_sb[:, 1:2], scale=1.0,
                                 accum_out=red_sb[:, 2 + jh:3 + jh])
        nc.vector.tensor_tensor(red_sb[:, 2:3], red_sb[:, 2:3],
                                red_sb[:, 3:4], op=ALU.add)
        nc.vector.reciprocal(red_sb[:, 4:5], red_sb[:, 2:3])

        # ------- h' = (ex @ Wh) * recip + h ; LayerNorm (normalize late) ----
        with tc.tile_pool(name="ps_fin", bufs=4, space="PSUM") as pf:
            for t in range(8):
                tp_ps = pf.tile([128, 128], BF16, name="tp_ps", tag="tp")
                nc.tensor.transpose(tp_ps[:], ex_sb[:, t * 128:(t + 1) * 128],
                                    iden_sb[:])
                if t % 2 == 0:
                    nc.vector.tensor_copy(attnT_sb[:, t * 128:(t + 1) * 128],
                                          tp_ps[:])
                else:
                    nc.scalar.copy(attnT_sb[:, t * 128:(t + 1) * 128],
                                   tp_ps[:])
            hp_ps = pf.tile([R, D], F32, name="hp_ps", bufs=1)
            for t in range(8):
                nc.tensor.matmul(hp_ps[:], attnT_sb[:, t * 128:(t + 1) * 128],
                                 Wh_sb[:, t * D:(t + 1) * D],
                                 start=(t == 0), stop=(t == 7))
            nc.vector.tensor_scalar(hp_sb[:], hp_ps[:], red_sb[:, 4:5], None,
                                    op0=ALU.mult)
            nc.vector.tensor_tensor(hp_sb[:], hp_sb[:], hrows_sb[:],
                                    op=ALU.add)

        nc.vector.reduce_sum(red_sb[:, 5:6], hp_sb[:], axis=AX.X)
        nc.vector.tensor_scalar(red_sb[:, 6:7], red_sb[:, 5:6], 1.0 / D, None,
                                op0=ALU.mult)
        nc.vector.tensor_scalar(xm_sb[:], hp_sb[:], red_sb[:, 6:7], None,
                                op0=ALU.subtract)
        nc.vector.tensor_tensor(sq_sb[:], xm_sb[:], xm_sb[:], op=ALU.mult)
        nc.vector.reduce_sum(red_sb[:, 7:8], sq_sb[:], axis=AX.X)
        # rstd = exp(-0.5 * ln(var + eps))
        nc.vector.tensor_scalar(red_sb[:, 7:8], red_sb[:, 7:8], 1.0 / D,
                                LN_EPS, op0=ALU.mult, op1=ALU.add)
        nc.scalar.activation(red_sb[:, 0:1], red_sb[:, 7:8], AF.Ln)
        nc.scalar.activation(red_sb[:, 0:1], red_sb[:, 0:1], AF.Exp,
                             bias=0.0, scale=-0.5)
        nc.vector.tensor_scalar(xm_sb[:], xm_sb[:], red_sb[:, 0:1], None,
                                op0=ALU.mult)
        nc.vector.tensor_tensor(o_sb[:], xm_sb[:], lngr_sb[:], op=ALU.mult)
        nc.vector.tensor_tensor(o_sb[:], o_sb[:], lnbr_sb[:], op=ALU.add)
        nc.sync.dma_start(out_d, o_sb[:])

    nc.compile()
    return nc


def _host_prep(inputs):
    h = np.asarray(inputs["h"], np.float32)[0]            # [N, D]
    adj = np.asarray(inputs["adj"])[0]                    # [N, N] int32
    W = np.asarray(inputs["W"], np.float32)
    attn_w1 = np.asarray(inputs["attn_w1"], np.float32)
    attn_b1 = np.asarray(inputs["attn_b1"], np.float32)
    attn_w2 = np.asarray(inputs["attn_w2"], np.float32)
    edge_w = np.asarray(inputs["edge_w"], np.float32)
    edge_b = np.asarray(inputs["edge_b"], np.float32)
    ln_g = np.asarray(inputs["ln_g"], np.float32)
    ln_b = np.asarray(inputs["ln_b"], np.float32)

    A_i, A_j, A_e = attn_w1[:D], attn_w1[D:2 * D], attn_w1[2 * D:]
    E_i, E_j = edge_w[:D], edge_w[D:]
    w2 = attn_w2[:, 0]

    hT = np.ascontiguousarray(h.T)                        # [D, N]
    Mv = W @ A_j + ALPHA * (E_j @ A_e)

    def blockdiag(M):
        Z = np.zeros((2 * D, 2 * D), np.float32)
        Z[:D, :D] = M
        Z[D:, D:] = M
        return Z

    lhsT1p = blockdiag(0.8 * A_e)
    lhsT2u = np.zeros((2 * D, 16 * 32), np.float32)
    for q in range(16):
        lhsT2u[:D, q * 32 + 2 * q] = w2
        lhsT2u[D:, q * 32 + 2 * q + 1] = w2

    rep = {
        "hT_f": hT,
        "lhsT1p": lhsT1p.astype(ml_dtypes.bfloat16),
        "lhsT2u": lhsT2u.astype(ml_dtypes.bfloat16),
        "ej2w": np.ascontiguousarray(np.concatenate([E_j, E_j], axis=1)),
        "v2w": np.ascontiguousarray(np.concatenate([Mv, Mv], axis=1)),
        "Ei2": blockdiag(E_i),
        "W2": blockdiag(W),
        "Ai2": blockdiag(A_i),
        "Ae2": blockdiag(A_e),
        "Wm": W,
        "b12col": np.concatenate([attn_b1, attn_b1])[:, None].copy(),
        "eb2col": np.concatenate([edge_b, edge_b])[:, None].copy(),
        "iden": np.eye(128, dtype=np.float32).astype(ml_dtypes.bfloat16),
        "lngr": np.broadcast_to(ln_g, (R, D)).copy(),
        "lnbr": np.broadcast_to(ln_b, (R, D)).copy(),
    }
    in_maps = []
    for c in range(NCORES):
        rows = slice(c * R, (c + 1) * R)
        hTc = hT[:, rows]                                 # [D, R]
        hTr2 = np.concatenate([hTc[:, 0::2], hTc[:, 1::2]], axis=0)  # [2D, P]
        m = dict(rep)
        m["hTr2"] = np.ascontiguousarray(hTr2)
        m["hrows"] = np.ascontiguousarray(h[rows])
        m["adjbig"] = ((adj[rows] - 1.0) * 1e9).astype(np.float32)
        in_maps.append(m)
    return in_maps


def _get_nc():
    if "nc" not in _CACHE:
        _CACHE["nc"] = _build_program()
    return _CACHE["nc"]


def kernel(**inputs) -> np.ndarray:
    nc = _get_nc()
    in_maps = _host_prep(inputs)
    res = run_bass_kernel_spmd(nc, in_maps, list(range(NCORES))).results
    out = np.concatenate([res[c]["out"] for c in range(NCORES)], axis=0)
    # rows were processed pair-interleaved: out row order is [0,2,4,...,1,3,...]
    # per core? No: bank row 2q <- pair q row i=2q (even), 2q+1 <- odd. e_sb
    # rows are already in natural order (row 2q of a 32-row group is node
    # grp*32+2q). Natural order preserved.
    return out[None].astype(np.float32)


# revision 44
# speedup vs baseline: 1.2308x; 1.0018x over previous
"""Trainium2 Bass kernel for EnhancedGraphAttentionLayer (B=1, N=1024, D=64).

Sharding: destination-node rows split across 8 cores (128 rows each).
Each core is fully independent (no collectives): it holds h replicated and
computes its 128 rows of scores/softmax/attention locally.

Row-paired formulation (2 destination rows per matmul stream):
  pre_i = 0.8*A_e^T relu(ej + ei_i + b) + V + u_i, with V = Mv^T hT constant
  across i (Mv = W@A_j + 0.2*E_j@A_e) and u_i = A_i^T Wh_i + b1
  + 0.2*A_e^T(ei_i + b) per-row.
  - stage1 (ACT): relu(ejT2 + eib2_col) -> rhs1 [128, N]; ejT2 holds ej^T
    duplicated in both partition halves, so one op covers rows i and i+1.
  - mm1 (PE): blockdiag(0.8A_e, 0.8A_e)^T @ rhs1 -> psum1 [128, N]: rows
    0:64 belong to row i, 64:128 to row i+1.
  - stage2 (custom DVE): rhs2 = LeakyRelu(psum1 + V2 + u2_col), V2 = [V; V].
  - mm2 (PE): paired one-hot lhsT accumulates e rows for i and i+1 into a
    32-row PSUM bank (w2 columns at bank rows 2q / 2q+1).
  e = w2^T LeakyRelu(pre) exactly (u rides inside the nonlinearity).
  Mask: e += (adj-1)*1e9, then softmax, attn @ Wh + h, LayerNorm.
The loop is software-pipelined with lag 2 (mm1 of pair p+2 issues before
mm2 of pair p) so the PE never waits on the DVE stage2. Wh is computed
after the main loop (PE fills the softmax-reduction window); the final
ex-transpose + attn @ Wh stack runs in bf16 and normalization by the
softmax denominator happens after the matmul, on [R, D] instead of [R, N].
All activations are pinned to one ACT table set (exp/ln/relu) so the
kernel pays a single ACT_TABLE_LOAD.
"""
import sys
import numpy as np

if "/opt/trn_rl_repo" not in sys.path:
    sys.path.insert(0, "/opt/trn_rl_repo")

import ml_dtypes
import concourse.bass as bass
import concourse.bacc as bacc
import concourse.mybir as mybir
import concourse.tile as tile
from concourse.bass_utils import run_bass_kernel_spmd
from concourse.dve_spec import Spec, Src0, Src1, C0, C2, lower, maxx
from concourse.dve_uop import DveOpSpec
from concourse.dve_ops import (DveOp, OPS, CUSTOM_DVE_SPECS,
                               _SUB_OPCODE_FOR_NAME, _CUSTOM_DVE_ROW_BASE)

F32 = mybir.dt.float32
F32R = mybir.dt.float32r
BF16 = mybir.dt.bfloat16
AF = mybir.ActivationFunctionType
ALU = mybir.AluOpType
AX = mybir.AxisListType

# Pin Relu/Exp/Ln to the one act-table set that holds all three
# ("natural_log_exp_and_others"), so the kernel needs a single
# ACT_TABLE_LOAD instead of mid-kernel table swaps (~1.3 us each + drain).
import concourse.hw_specs as _hw
import concourse.bacc as _bacc_mod


def _pin_act_tables():
    if getattr(_hw, "_act_tables_pinned", False):
        return
    orig = _hw.get_activation_tables

    import functools

    @functools.cache
    def pinned(arch):
        t = dict(orig(arch))
        keep = "natural_log_exp_and_others"
        if keep not in t:
            return t
        pin_funcs = {AF.Relu, AF.Exp, AF.Ln} & t[keep]
        return {name: (funcs if name == keep else funcs - pin_funcs)
                for name, funcs in t.items()}

    _hw.get_activation_tables = pinned
    _bacc_mod.get_activation_tables = pinned
    _hw._act_tables_pinned = True


_pin_act_tables()

N = 1024
D = 64
NCORES = 8
R = N // NCORES          # 128 rows per core
P = R // 2               # 64 row-pairs per core
ALPHA = 0.2
LN_EPS = 1e-5

_CACHE = {}


def _register_dve_op(name, spec):
    if name in _SUB_OPCODE_FOR_NAME:
        return next(op for op in OPS if op.name == name)
    shas = {}
    for ver in ("v3", "v4"):
        shas[ver] = DveOpSpec(name=name, uops=lower(spec, ver=ver), opcode=0,
                              rd1_en=True).sha(ver)
    op = DveOp(name, spec, subdim=False, uops_sha=shas)
    OPS.append(op)
    row = _CUSTOM_DVE_ROW_BASE + len(OPS) - 1
    assert row < 0x20
    _SUB_OPCODE_FOR_NAME[name] = row
    CUSTOM_DVE_SPECS[name] = spec
    return op


def _register_lrelu_vb():
    """out = LeakyRelu(in0 + in1 + s0) with slope imm2, registered at runtime."""
    y = Src0 + Src1 + C0
    return _register_dve_op("LRELU_VB_ANT", Spec(
        body=maxx(y, y * C2),
        reference=lambda in0, in1, s0, s1, imm2: np.maximum(
            in0 + in1 + s0, (in0 + in1 + s0) * imm2),
    ))


def _build_program():
    lrelu_vb = _register_lrelu_vb()
    nc = bacc.Bacc("TRN2", target_bir_lowering=False, debug=False,
                   num_devices=NCORES)

    def din(name, shape, dt):
        return nc.dram_tensor(name, shape, dt, kind="ExternalInput").ap()

    hT_f = din("hT_f", [D, N], F32R)
    hTr2 = din("hTr2", [2 * D, P], F32)
    hrows = din("hrows", [R, D], F32)
    adjbig = din("adjbig", [R, N], F32)
    lhsT1p = din("lhsT1p", [2 * D, 2 * D], BF16)   # blockdiag(0.8Ae, 0.8Ae)
    lhsT2u = din("lhsT2u", [2 * D, 16 * 32], BF16)  # paired one-hot w2 cols
    ej2w = din("ej2w", [D, 2 * D], F32R)           # [Ej | Ej]
    v2w = din("v2w", [D, 2 * D], F32R)             # [Mv | Mv]
    Ei2 = din("Ei2", [2 * D, 2 * D], F32)          # blockdiag(Ei, Ei)
    W2 = din("W2", [2 * D, 2 * D], F32)            # blockdiag(W, W)
    Ai2 = din("Ai2", [2 * D, 2 * D], F32)          # blockdiag(Ai, Ai)
    Ae2 = din("Ae2", [2 * D, 2 * D], F32)          # blockdiag(Ae, Ae)
    Wm = din("Wm", [D, D], F32R)
    b12col = din("b12col", [2 * D, 1], F32)
    eb2col = din("eb2col", [2 * D, 1], F32)
    iden = din("iden", [128, 128], BF16)
    lngr = din("lngr", [R, D], F32)
    lnbr = din("lnbr", [R, D], F32)
    out_d = nc.dram_tensor("out", [R, D], F32, kind="ExternalOutput").ap()

    with tile.TileContext(nc) as tc, \
         tc.tile_pool(name="static", bufs=1) as sp:
        # ---------------- static SBUF tiles ----------------
        hT_sb = sp.tile([D, N], F32R, name="hT_sb", tag="hT_sb")
        hTr2_sb = sp.tile([2 * D, P], F32, name="hTr2_sb", tag="hTr2_sb")
        hrows_sb = sp.tile([R, D], F32, name="hrows_sb", tag="hrows_sb")
        adjb_sb = sp.tile([R, N], F32, name="adjb_sb", tag="adjb_sb")
        lhsT1p_sb = sp.tile([2 * D, 2 * D], BF16, name="lhsT1p_sb", tag="l1")
        lhsT2u_sb = sp.tile([2 * D, 16 * 32], BF16, name="lhsT2u_sb", tag="l2")
        ej2w_sb = sp.tile([D, 2 * D], F32R, name="ej2w_sb", tag="ej2w")
        v2w_sb = sp.tile([D, 2 * D], F32R, name="v2w_sb", tag="v2w")
        Ei2_sb = sp.tile([2 * D, 2 * D], F32, name="Ei2_sb", tag="Ei2")
        W2_sb = sp.tile([2 * D, 2 * D], F32, name="W2_sb", tag="W2")
        Ai2_sb = sp.tile([2 * D, 2 * D], F32, name="Ai2_sb", tag="Ai2")
        Ae2_sb = sp.tile([2 * D, 2 * D], F32, name="Ae2_sb", tag="Ae2")
        Wm_sb = sp.tile([D, D], F32R, name="Wm_sb", tag="Wm")
        b12_sb = sp.tile([2 * D, 1], F32, name="b12_sb", tag="b12")
        eb2_sb = sp.tile([2 * D, 1], F32, name="eb2_sb", tag="eb2")
        iden_sb = sp.tile([128, 128], BF16, name="iden_sb", tag="iden")
        lngr_sb = sp.tile([R, D], F32, name="lngr_sb", tag="lngr")
        lnbr_sb = sp.tile([R, D], F32, name="lnbr_sb", tag="lnbr")

        ejT2_sb = sp.tile([2 * D, N], BF16, name="ejT2_sb", tag="ejT2")
        V2_sb = sp.tile([2 * D, N], F32, name="V2_sb", tag="V2")
        eib2_sb = sp.tile([2 * D, P], F32, name="eib2_sb", tag="eib2")
        WhTr2_sb = sp.tile([2 * D, P], F32, name="WhTr2_sb", tag="WhTr2")
        q2_sb = sp.tile([2 * D, P], F32, name="q2_sb", tag="q2")
        u2_sb = sp.tile([2 * D, P], F32, name="u2_sb", tag="u2")
        Wh_sb = sp.tile([128, 8 * D], BF16, name="Wh_sb", tag="Wh")
        # 3 slots each: the lag-2 software pipeline keeps 3 pairs in flight
        rhs1_sb = sp.tile([2 * D, 3 * N], BF16, name="rhs1_sb", tag="rhs1")
        rhs2_sb = sp.tile([2 * D, 3 * N], BF16, name="rhs2_sb", tag="rhs2")
        e_sb = sp.tile([R, N], F32, name="e_sb", tag="e_sb")
        ex_sb = sp.tile([R, N], BF16, name="ex_sb", tag="ex_sb")
        attnT_sb = sp.tile([128, N], BF16, name="attnT_sb", tag="attnT")
        scr_sb = sp.tile([1, 8], F32, name="scr_sb", tag="scr")
        red_sb = sp.tile([R, 10], F32, name="red_sb", tag="red")
        hp_sb = sp.tile([R, D], F32, name="hp_sb", tag="hp")
        xm_sb = sp.tile([R, D], F32, name="xm_sb", tag="xm")
        sq_sb = sp.tile([R, D], F32, name="sq_sb", tag="sq")
        o_sb = sp.tile([R, D], F32, name="o_sb", tag="o")

        # ------------- load inputs (critical-path tensors first) -------------
        # critical path (gates ej2/eib2/stage1/mm1) on the sync queue alone
        nc.sync.dma_start(hT_sb[:], hT_f)
        nc.sync.dma_start(ej2w_sb[:], ej2w)
        nc.sync.dma_start(hTr2_sb[:], hTr2)
        nc.sync.dma_start(Ei2_sb[:], Ei2)
        nc.sync.dma_start(eb2_sb[:], eb2col)
        nc.sync.dma_start(lhsT1p_sb[:], lhsT1p)
        # everything else via the otherwise-idle GpSimd queue
        nc.gpsimd.dma_start(v2w_sb[:], v2w)
        nc.gpsimd.dma_start(W2_sb[:], W2)
        nc.gpsimd.dma_start(Ai2_sb[:], Ai2)
        nc.gpsimd.dma_start(Ae2_sb[:], Ae2)
        nc.gpsimd.dma_start(b12_sb[:], b12col)
        nc.gpsimd.dma_start(lhsT2u_sb[:], lhsT2u)
        nc.gpsimd.dma_start(Wm_sb[:], Wm)
        nc.gpsimd.dma_start(adjb_sb[:], adjbig)
        nc.gpsimd.dma_start(hrows_sb[:], hrows)
        nc.gpsimd.dma_start(iden_sb[:], iden)
        nc.gpsimd.dma_start(lngr_sb[:], lngr)
        nc.gpsimd.dma_start(lnbr_sb[:], lnbr)

        # warm ACT table sets early (exp/ln)
        nc.vector.memset(scr_sb[:], 1.0)
        nc.scalar.activation(scr_sb[0:1, 0:1], scr_sb[0:1, 1:2], AF.Exp)
        nc.scalar.activation(scr_sb[0:1, 2:3], scr_sb[0:1, 3:4], AF.Ln)

        # ---------------- setup math ----------------
        with tc.tile_pool(name="ps_setup", bufs=1, space="PSUM") as psp:
            # ejT2 = [Ej|Ej]^T hT (bf16), V2 = [Mv|Mv]^T hT (f32)
            for jh in range(2):
                ej_ps = psp.tile([2 * D, 512], F32, name="ej_ps", tag="big",
                                 bufs=2)
                nc.tensor.matmul(ej_ps[:], ej2w_sb[:],
                                 hT_sb[:, jh * 512:(jh + 1) * 512])
                nc.vector.tensor_copy(ejT2_sb[:, jh * 512:(jh + 1) * 512],
                                      ej_ps[:])
            # eib2 = blockdiag(Ei,Ei)^T hTr2 + eb2 (gates stage1(0) - early)
            eib_ps = psp.tile([2 * D, P], F32, name="eib_ps", tag="small",
                              bufs=2)
            nc.tensor.matmul(eib_ps[:], Ei2_sb[:], hTr2_sb[:])
            nc.vector.tensor_scalar(eib2_sb[:], eib_ps[:], eb2_sb[:], None,
                                    op0=ALU.add)
            for jh in range(2):
                v_ps = psp.tile([2 * D, 512], F32, name="v_ps", tag="big",
                                bufs=2)
                nc.tensor.matmul(v_ps[:], v2w_sb[:],
                                 hT_sb[:, jh * 512:(jh + 1) * 512])
                nc.vector.tensor_copy(V2_sb[:, jh * 512:(jh + 1) * 512],
                                      v_ps[:])
            # WhTr2 = blockdiag(W,W)^T hTr2 (paired row projections)
            whtr_ps = psp.tile([2 * D, P], F32, name="whtr_ps", tag="small",
                               bufs=2)
            nc.tensor.matmul(whtr_ps[:], W2_sb[:], hTr2_sb[:])
            nc.vector.tensor_copy(WhTr2_sb[:], whtr_ps[:])
            # q2 = blockdiag(Ai,Ai)^T WhTr2 + b12
            q_ps = psp.tile([2 * D, P], F32, name="q_ps", tag="small", bufs=2)
            nc.tensor.matmul(q_ps[:], Ai2_sb[:], WhTr2_sb[:])
            nc.vector.tensor_scalar(q2_sb[:], q_ps[:], b12_sb[:], None,
                                    op0=ALU.add)
            # u2 = q2 + 0.2 * blockdiag(Ae,Ae)^T eib2
            z_ps = psp.tile([2 * D, P], F32, name="z_ps", tag="small", bufs=2)
            nc.tensor.matmul(z_ps[:], Ae2_sb[:], eib2_sb[:])
            nc.vector.scalar_tensor_tensor(
                u2_sb[:], z_ps[:], ALPHA, q2_sb[:], op0=ALU.mult, op1=ALU.add)
            # first half of Wh here: PE filler while stage1(0) runs on ACT
            for t in range(4):
                wh_ps = psp.tile([128, D], F32, name="wh_ps", tag="small",
                                 bufs=2)
                nc.tensor.matmul(wh_ps[:], hT_sb[:, t * 128:(t + 1) * 128],
                                 Wm_sb[:])
                nc.vector.tensor_copy(Wh_sb[:, t * D:(t + 1) * D], wh_ps[:])

        # ---- main loop over 64 row-pairs (lag-2 software pipeline) ----
        # PE program order: mm1(p+2) precedes mm2(p), so stage2(p) on the DVE
        # has two full mm1 windows to finish before the PE needs its output.
        def stage1(p):
            buf = p % 3
            nc.scalar.activation(rhs1_sb[:, buf * N:(buf + 1) * N],
                                 ejT2_sb[:], AF.Relu,
                                 bias=eib2_sb[:, p:p + 1], scale=1.0)

        with tc.tile_pool(name="ps_mm1", bufs=2, space="PSUM") as pmm1, \
             tc.tile_pool(name="ps_e", bufs=4, space="PSUM") as pe:
            psum1 = [None, None]

            def mm1(p):
                s = p % 3
                psum1[p % 2] = pmm1.tile([2 * D, N], F32, name="psum1",
                                         tag="psum1")
                for jh in range(2):
                    nc.tensor.matmul(
                        psum1[p % 2][:, jh * 512:(jh + 1) * 512],
                        lhsT1p_sb[:],
                        rhs1_sb[:, s * N + jh * 512: s * N + (jh + 1) * 512])

            def stage2(p):
                s = p % 3
                nc.vector._custom_dve(
                    lrelu_vb,
                    out=rhs2_sb[:, s * N:(s + 1) * N],
                    in0=psum1[p % 2][:], in1=V2_sb[:],
                    s0=u2_sb[:, p:p + 1], imm2=ALPHA)

            banks = {}      # group -> [bankE_jh0, bankE_jh1]
            stage1(0)
            mm1(0)
            stage1(1)
            mm1(1)
            stage2(0)
            for p in range(P):
                q = p % 16
                grp = p // 16
                buf = p % 3
                if q == 0:
                    banks[grp] = [pe.tile([32, 512], F32, name="bankE",
                                          tag="bankE") for _ in range(2)]
                if p + 2 < P:
                    stage1(p + 2)
                    mm1(p + 2)
                if p + 1 < P:
                    stage2(p + 1)
                # score matmul: accumulate e rows (2q, 2q+1) into banks
                for jh in range(2):
                    nc.tensor.matmul(
                        banks[grp][jh][:],
                        lhsT2u_sb[:, q * 32:(q + 1) * 32],
                        rhs2_sb[:, buf * N + jh * 512: buf * N + (jh + 1) * 512],
                        start=(q == 0), stop=(q == 15))
                if q == 15:
                    # drain bank -> e_sb on the Scalar engine: the DVE is
                    # pacing stage2, and a DVE drain here stalls the PE
                    # ~1.5us at every group boundary
                    for jh in range(2):
                        rows = slice(grp * 32, (grp + 1) * 32)
                        nc.scalar.copy(e_sb[rows, jh * 512:(jh + 1) * 512],
                                       banks[grp][jh][:])

        # ---------------- mask + softmax ----------------
        nc.vector.tensor_tensor(e_sb[:], e_sb[:], adjb_sb[:], op=ALU.add)
        nc.vector.reduce_max(red_sb[:, 0:1], e_sb[:], axis=AX.X)
        nc.vector.tensor_scalar(red_sb[:, 1:2], red_sb[:, 0:1], -1.0, None,
                                op0=ALU.mult)

        # Wh node-major [128, 64] x 8 tiles — emitted after the main loop so
        # the PE computes it during the softmax reductions (Wh is only
        # needed by the final attn @ Wh).
        with tc.tile_pool(name="ps_wh", bufs=2, space="PSUM") as pw:
            for t in range(4, 8):
                wh_ps = pw.tile([128, D], F32, name="wh_ps", tag="wh", bufs=2)
                nc.tensor.matmul(wh_ps[:], hT_sb[:, t * 128:(t + 1) * 128],
                                 Wm_sb[:])
                nc.vector.tensor_copy(Wh_sb[:, t * D:(t + 1) * D], wh_ps[:])

        # exp in two halves so the first transposes can start earlier
        for jh in range(2):
            nc.scalar.activation(ex_sb[:, jh * 512:(jh + 1) * 512],
                                 e_sb[:, jh * 512:(jh + 1) * 512], AF.Exp,
                                 bias=red_sb[:, 1:2], scale=1.0,
                                 accum_out=red_sb[:, 2 + jh:3 + jh])
        nc.vector.tensor_tensor(red_sb[:, 2:3], red_sb[:, 2:3],
                                red_sb[:, 3:4], op=ALU.add)
        nc.vector.reciprocal(red_sb[:, 4:5], red_sb[:, 2:3])

        # ------- h' = (ex @ Wh) * recip + h ; LayerNorm (normalize late) ----
        with tc.tile_pool(name="ps_fin", bufs=4, space="PSUM") as pf:
            for t in range(8):
                tp_ps = pf.tile([128, 128], BF16, name="tp_ps", tag="tp")
                nc.tensor.transpose(tp_ps[:], ex_sb[:, t * 128:(t + 1) * 128],
                                    iden_sb[:])
                if t % 2 == 0:
                    nc.vector.tensor_copy(attnT_sb[:, t * 128:(t + 1) * 128],
                                          tp_ps[:])
                else:
                    nc.scalar.copy(attnT_sb[:, t * 128:(t + 1) * 128],
                                   tp_ps[:])
            hp_ps = pf.tile([R, D], F32, name="hp_ps", bufs=1)
            for t in range(8):
                nc.tensor.matmul(hp_ps[:], attnT_sb[:, t * 128:(t + 1) * 128],
                                 Wh_sb[:, t * D:(t + 1) * D],
                                 start=(t == 0), stop=(t == 7))
            nc.vector.tensor_scalar(hp_sb[:], hp_ps[:], red_sb[:, 4:5], None,
                                    op0=ALU.mult)
            nc.vector.tensor_tensor(hp_sb[:], hp_sb[:], hrows_sb[:],
                                    op=ALU.add)

        nc.vector.reduce_sum(red_sb[:, 5:6], hp_sb[:], axis=AX.X)
        nc.vector.tensor_scalar(red_sb[:, 6:7], red_sb[:, 5:6], 1.0 / D, None,
                                op0=ALU.mult)
        nc.vector.tensor_scalar(xm_sb[:], hp_sb[:], red_sb[:, 6:7], None,
                                op0=ALU.subtract)
        nc.vector.tensor_tensor(sq_sb[:], xm_sb[:], xm_sb[:], op=ALU.mult)
        nc.vector.reduce_sum(red_sb[:, 7:8], sq_sb[:], axis=AX.X)
        # rstd = exp(-0.5 * ln(var + eps))
        nc.vector.tensor_scalar(red_sb[:, 7:8], red_sb[:, 7:8], 1.0 / D,
                                LN_EPS, op0=ALU.mult, op1=ALU.add)
        nc.scalar.activation(red_sb[:, 0:1], red_sb[:, 7:8], AF.Ln)
        nc.scalar.activation(red_sb[:, 0:1], red_sb[:, 0:1], AF.Exp,
                             bias=0.0, scale=-0.5)
        nc.vector.tensor_scalar(xm_sb[:], xm_sb[:], red_sb[:, 0:1], None,
                                op0=ALU.mult)
        nc.vector.tensor_tensor(o_sb[:], xm_sb[:], lngr_sb[:], op=ALU.mult)
        nc.vector.tensor_tensor(o_sb[:], o_sb[:], lnbr_sb[:], op=ALU.add)
        nc.sync.dma_start(out_d, o_sb[:])

    nc.compile()
    return nc


def _host_prep(inputs):
    h = np.asarray(inputs["h"], np.float32)[0]            # [N, D]
    adj = np.asarray(inputs["adj"])[0]                    # [N, N] int32
    W = np.asarray(inputs["W"], np.float32)
    attn_w1 = np.asarray(inputs["attn_w1"], np.float32)
    attn_b1 = np.asarray(inputs["attn_b1"], np.float32)
    attn_w2 = np.asarray(inputs["attn_w2"], np.float32)
    edge_w = np.asarray(inputs["edge_w"], np.float32)
    edge_b = np.asarray(inputs["edge_b"], np.float32)
    ln_g = np.asarray(inputs["ln_g"], np.float32)
    ln_b = np.asarray(inputs["ln_b"], np.float32)

    A_i, A_j, A_e = attn_w1[:D], attn_w1[D:2 * D], attn_w1[2 * D:]
    E_i, E_j = edge_w[:D], edge_w[D:]
    w2 = attn_w2[:, 0]

    hT = np.ascontiguousarray(h.T)                        # [D, N]
    Mv = W @ A_j + ALPHA * (E_j @ A_e)

    def blockdiag(M):
        Z = np.zeros((2 * D, 2 * D), np.float32)
        Z[:D, :D] = M
        Z[D:, D:] = M
        return Z

    lhsT1p = blockdiag(0.8 * A_e)
    lhsT2u = np.zeros((2 * D, 16 * 32), np.float32)
    for q in range(16):
        lhsT2u[:D, q * 32 + 2 * q] = w2
        lhsT2u[D:, q * 32 + 2 * q + 1] = w2

    rep = {
        "hT_f": hT,
        "lhsT1p": lhsT1p.astype(ml_dtypes.bfloat16),
        "lhsT2u": lhsT2u.astype(ml_dtypes.bfloat16),
        "ej2w": np.ascontiguousarray(np.concatenate([E_j, E_j], axis=1)),
        "v2w": np.ascontiguousarray(np.concatenate([Mv, Mv], axis=1)),
        "Ei2": blockdiag(E_i),
        "W2": blockdiag(W),
        "Ai2": blockdiag(A_i),
        "Ae2": blockdiag(A_e),
        "Wm": W,
        "b12col": np.concatenate([attn_b1, attn_b1])[:, None].copy(),
        "eb2col": np.concatenate([edge_b, edge_b])[:, None].copy(),
        "iden": np.eye(128, dtype=np.float32).astype(ml_dtypes.bfloat16),
        "lngr": np.broadcast_to(ln_g, (R, D)).copy(),
        "lnbr": np.broadcast_to(ln_b, (R, D)).copy(),
    }
    in_maps = []
    for c in range(NCORES):
        rows = slice(c * R, (c + 1) * R)
        hTc = hT[:, rows]                                 # [D, R]
        hTr2 = np.concatenate([hTc[:, 0::2], hTc[:, 1::2]], axis=0)  # [2D, P]
        m = dict(rep)
        m["hTr2"] = np.ascontiguousarray(hTr2)
        m["hrows"] = np.ascontiguousarray(h[rows])
        m["adjbig"] = ((adj[rows] - 1.0) * 1e9).astype(np.float32)
        in_maps.append(m)
    return in_maps


def _get_nc():
    if "nc" not in _CACHE:
        _CACHE["nc"] = _build_program()
    return _CACHE["nc"]


def kernel(**inputs) -> np.ndarray:
    nc = _get_nc()
    in_maps = _host_prep(inputs)
    res = run_bass_kernel_spmd(nc, in_maps, list(range(NCORES))).results
    out = np.concatenate([res[c]["out"] for c in range(NCORES)], axis=0)
    # rows were processed pair-interleaved: out row order is [0,2,4,...,1,3,...]
    # per core? No: bank row 2q <- pair q row i=2q (even), 2q+1 <- odd. e_sb
    # rows are already in natural order (row 2q of a 32-row group is node
    # grp*32+2q). Natural order preserved.
    return out[None].astype(np.float32)
